# revision 9
# baseline (speedup 1.0000x reference)
"""Trainium2 Bass kernel for the AdapterModel problem.

Data-parallel over batch: core b computes pred[b] = f(seq_embed[b], aa_embed[b], ...).
No collectives needed (B == n_cores == 8); host gathers per-core outputs.

Math per core (N=896 seq positions, J=512 aa positions, H=32 heads, D=64):
  seq_lat^T[e,i] = seq_w^T @ seq_embed^T + seq_b     (e = h*64+d, PE, f32r)
  aa_lat^T[e,j]  = aa_w^T @ aa_embed^T + aa_b
  aa_norm = aa_lat * rsqrt(sum_d aa_lat^2) * mask    (norms via pairmask matmul)
  G[h,i,j] = <seq_lat_h[:,i], aa_norm_h[:,j]>        (K=64 matmuls)
  s[i,h] = sum_j exp(G * (100*rsqrt(nsq_seq))_i - 40)    (ACT exp + accum_out)
  r[i,h] = 0.01*ln(s) + 0.01*(40 - 2*ln n)   (reference's logavgexp subtracts ln n twice)
  gating: wv = (to_logits_w .* sigmoid(ctx @ ctx_w + ctx_b)) @ pred_w
  out[i] = softplus(r[i,:] @ wv + pred_b)
All transcendentals use only the exp/ln ACT table set (rsqrt = exp(-0.5*ln),
sigmoid/softplus via exp + ln/reciprocal) to avoid table-set thrash.
"""
import sys

if "/opt/trn_rl_repo" not in sys.path:
    sys.path.insert(0, "/opt/trn_rl_repo")

import math
import numpy as np

H = 32
D = 64
E = H * D            # 2048
SEQ_D = 3072
AA_D = 1280
CTX_D = 768
B, N, J = 8, 896, 512
KS = SEQ_D // 128    # 24
KA = AA_D // 128     # 10
EB = E // 128        # 16
IB = N // 128        # 7
NH = N // 2          # 448

_GRAPH_CACHE = {}


def _build(pred_b_val: float, stage: str = "full"):
    key = (float(pred_b_val), stage)
    if key in _GRAPH_CACHE:
        return _GRAPH_CACHE[key]

    import concourse.bacc as bacc
    import concourse.mybir as mybir
    import concourse.tile as tile

    F32 = mybir.dt.float32
    F32R = mybir.dt.float32r
    AF = mybir.ActivationFunctionType
    AL = mybir.AluOpType

    nc = bacc.Bacc("TRN2", target_bir_lowering=False, debug=False, num_devices=8)

    seq_ext = nc.dram_tensor("seq", [N, SEQ_D], F32, kind="ExternalInput")
    aa_ext = nc.dram_tensor("aa", [J, AA_D], F32, kind="ExternalInput")
    seqw_ext = nc.dram_tensor("seq_w", [SEQ_D, E], F32, kind="ExternalInput")
    aaw_ext = nc.dram_tensor("aa_w", [AA_D, E], F32, kind="ExternalInput")
    seqb_ext = nc.dram_tensor("seq_b2", [128, EB], F32, kind="ExternalInput")
    aab_ext = nc.dram_tensor("aa_b2", [128, EB], F32, kind="ExternalInput")
    ctxT_ext = nc.dram_tensor("ctxT", [128, CTX_D // 128], F32, kind="ExternalInput")
    ctxw_ext = nc.dram_tensor("ctx_wp", [CTX_D, H * H], F32, kind="ExternalInput")
    ctxb_ext = nc.dram_tensor("ctx_bp", [1, H * H], F32, kind="ExternalInput")
    tlw_ext = nc.dram_tensor("tlwT", [1, H * H], F32, kind="ExternalInput")
    predw_ext = nc.dram_tensor("pred_w", [H, 1], F32, kind="ExternalInput")
    mask2_ext = nc.dram_tensor("mask2", [2, J], F32, kind="ExternalInput")
    cvec_ext = nc.dram_tensor("cvec", [128, 1], F32, kind="ExternalInput")
    eye_ext = nc.dram_tensor("eye128", [128, 128], F32, kind="ExternalInput")
    eye2_ext = nc.dram_tensor("eye2", [2, 2], F32, kind="ExternalInput")
    pm_ext = nc.dram_tensor("pairmask", [128, 2], F32, kind="ExternalInput")
    sel2_ext = nc.dram_tensor("sel2", [2, 128], F32, kind="ExternalInput")
    ones1_ext = nc.dram_tensor("ones1", [1, 128], F32, kind="ExternalInput")
    out_ext = nc.dram_tensor("out", [N], F32, kind="ExternalOutput")

    LN100 = math.log(100.0)
    KCTX = CTX_D // 128  # 6

    with tile.TileContext(nc) as tc:
        with tc.tile_pool(name="persist", bufs=1) as pp, \
             tc.tile_pool(name="work", bufs=2) as wp, \
             tc.tile_pool(name="pproj", bufs=3, space="PSUM") as pproj, \
             tc.tile_pool(name="pb512", bufs=2, space="PSUM") as pb512, \
             tc.tile_pool(name="pips", bufs=2, space="PSUM") as pips, \
             tc.tile_pool(name="psm2", bufs=1, space="PSUM") as psm2:

            def dummy_out(src_ap):
                dummy = pp.tile([128, 1], F32, tag="dummy", name="dummy")
                nc.vector.tensor_copy(dummy[:], src_ap)
                for ib in range(IB):
                    nc.sync.dma_start(out=out_ext[128 * ib:128 * (ib + 1)], in_=dummy[:])

            # ---- constants / tiny inputs ----
            eye = pp.tile([128, 128], F32, tag="eye")
            nc.sync.dma_start(out=eye[:], in_=eye_ext[:])
            eye2 = pp.tile([2, 2], F32, tag="eye2")
            nc.sync.dma_start(out=eye2[:], in_=eye2_ext[:])
            pairmask = pp.tile([128, 2], F32R, tag="pairmask")
            nc.gpsimd.dma_start(out=pairmask[:], in_=pm_ext[:])
            sel2 = pp.tile([2, 128], F32R, tag="sel2")
            nc.gpsimd.dma_start(out=sel2[:], in_=sel2_ext[:])
            ones1 = pp.tile([1, 128], F32R, tag="ones1")
            nc.gpsimd.dma_start(out=ones1[:], in_=ones1_ext[:])
            mask2 = pp.tile([2, J], F32, tag="mask2")
            nc.sync.dma_start(out=mask2[:], in_=mask2_ext[:])
            cvec = pp.tile([128, 1], F32, tag="cvec")
            nc.sync.dma_start(out=cvec[:], in_=cvec_ext[:])
            seqb2 = pp.tile([128, EB], F32, tag="seqb2")
            nc.sync.dma_start(out=seqb2[:], in_=seqb_ext[:])
            aab2 = pp.tile([128, EB], F32, tag="aab2")
            nc.sync.dma_start(out=aab2[:], in_=aab_ext[:])
            # const bias columns: [0]=-40, [1]=1e-30, [2]=ln(100), [3]=pred_b
            cb = pp.tile([128, 4], F32, tag="cb")
            nc.gpsimd.memset(cb[:, 0:1], -40.0)
            nc.gpsimd.memset(cb[:, 1:2], 1e-30)
            nc.gpsimd.memset(cb[:, 2:3], LN100)
            nc.gpsimd.memset(cb[:, 3:4], float(pred_b_val))

            # ---- gating chain ----
            ctxT = pp.tile([128, KCTX], F32R, tag="ctxT")
            nc.gpsimd.dma_start(out=ctxT[:], in_=ctxT_ext[:])
            g_ps = [pips.tile([1, 512], F32, tag="ips", name=f"gps{i}") for i in range(2)]
            for c in range(KCTX):
                wctx = wp.tile([128, H * H], F32R, tag="wseq", name=f"wctx{c}")
                nc.gpsimd.dma_start(out=wctx[:], in_=ctxw_ext[128 * c:128 * (c + 1), :])
                for half in range(2):
                    nc.tensor.matmul(g_ps[half][:], ctxT[:, c:c + 1], wctx[:, 512 * half:512 * (half + 1)],
                                     start=(c == 0), stop=(c == KCTX - 1))
            ctxb = pp.tile([1, H * H], F32, tag="ctxb")
            nc.sync.dma_start(out=ctxb[:], in_=ctxb_ext[:])
            g_sb = pp.tile([1, H * H], F32, tag="g_sb")
            for half in range(2):
                nc.vector.tensor_add(g_sb[:, 512 * half:512 * (half + 1)], g_ps[half][:], ctxb[:, 512 * half:512 * (half + 1)])
            # sigmoid(x) = 1/(1+exp(-x))
            sig = pp.tile([1, H * H], F32, tag="sig")
            nc.scalar.activation(sig[:], g_sb[:], AF.Exp, bias=0.0, scale=-1.0)
            nc.vector.tensor_scalar_add(sig[:], sig[:], 1.0)
            nc.vector.reciprocal(sig[:], sig[:])
            tlw = pp.tile([1, H * H], F32, tag="tlw")
            nc.sync.dma_start(out=tlw[:], in_=tlw_ext[:])
            nc.vector.tensor_mul(sig[:], sig[:], tlw[:])   # w_b^T flat, e-major
            wb_dram = nc.dram_tensor("wb_bounce", [H, H], F32)
            nc.sync.dma_start(out=wb_dram.ap().rearrange("e h -> (e h)")[None, :], in_=sig[:])
            wbT = pp.tile([H, H], F32R, tag="wbT")
            nc.gpsimd.dma_start(out=wbT[:], in_=wb_dram[:])
            predw = pp.tile([H, 1], F32R, tag="predw")
            nc.gpsimd.dma_start(out=predw[:], in_=predw_ext[:])
            wv_ps = pips.tile([1, H], F32, tag="ips")
            nc.tensor.matmul(wv_ps[:], predw[:], wbT[:], start=True, stop=True)
            wv_sb = pp.tile([1, H], F32R, tag="wv_sb")
            nc.vector.tensor_copy(wv_sb[:], wv_ps[:])
            WV_ps = pips.tile([128, H], F32, tag="ips")
            nc.tensor.matmul(WV_ps[:], ones1[:], wv_sb[:], start=True, stop=True)
            WV = pp.tile([128, H], F32, tag="WV")
            nc.vector.tensor_copy(WV[:], WV_ps[:])

            if stage == "gating":
                dummy_out(WV[:, 0:1])

            # ---- phase 0: transpose aa_embed and seq_embed ----
            if stage in ("tpose", "eb", "full"):
                aaT = [pp.tile([128, J], F32R, tag=f"aat{kb}", name=f"aat{kb}") for kb in range(KA)]
                for jb in range(J // 128):
                    nat = wp.tile([128, AA_D], F32, tag="nat", bufs=2, name=f"anat{jb}")
                    nc.sync.dma_start(out=nat[:], in_=aa_ext[128 * jb:128 * (jb + 1), :])
                    for kb in range(KA):
                        tp = pips.tile([128, 128], F32, tag="ips", name=f"atp{jb}_{kb}")
                        nc.tensor.transpose(tp[:], nat[:, 128 * kb:128 * (kb + 1)], eye[:])
                        nc.any.tensor_copy(aaT[kb][:, 128 * jb:128 * (jb + 1)], tp[:])

                seqT = [pp.tile([128, N], F32R, tag=f"ast{kb}", name=f"ast{kb}") for kb in range(KS)]
                for ib in range(IB):
                    for hf in range(2):
                        nat = wp.tile([128, SEQ_D // 2], F32, tag="nat", bufs=2, name=f"snat{ib}_{hf}")
                        nc.sync.dma_start(out=nat[:], in_=seq_ext[128 * ib:128 * (ib + 1), (SEQ_D // 2) * hf:(SEQ_D // 2) * (hf + 1)])
                        for k in range(KS // 2):
                            kb = hf * (KS // 2) + k
                            tp = pips.tile([128, 128], F32, tag="ips", name=f"stp{ib}_{kb}")
                            nc.tensor.transpose(tp[:], nat[:, 128 * k:128 * (k + 1)], eye[:])
                            nc.any.tensor_copy(seqT[kb][:, 128 * ib:128 * (ib + 1)], tp[:])

            if stage == "tpose":
                dummy_out(seqT[0][:, 0:1].bitcast(F32))

            # ---- phase 1: per head-pair block ----
            if stage in ("eb", "full"):
                invsT = [pp.tile([128, H], F32, tag=f"invsT{ib}", name=f"invsT{ib}") for ib in range(IB)]
                s_t = [pp.tile([128, H], F32, tag=f"s_t{ib}", name=f"s_t{ib}") for ib in range(IB)]

                for eb in range(EB):
                    # aa projection for this e-block
                    aa_ps = pproj.tile([128, J], F32, tag="proj", name=f"aaps{eb}")
                    waa = wp.tile([128, KA * 128], F32R, tag="waa", name=f"waa{eb}")
                    nc.gpsimd.dma_start(
                        out=waa[:],
                        in_=aaw_ext.ap().rearrange("(kb p) e -> p kb e", p=128)[:, :, 128 * eb:128 * (eb + 1)])
                    for kb in range(KA):
                        nc.tensor.matmul(aa_ps[:], waa[:, 128 * kb:128 * (kb + 1)], aaT[kb][:],
                                         start=(kb == 0), stop=(kb == KA - 1))
                    aa_raw = wp.tile([128, J], F32, tag="araw", name=f"araw{eb}")
                    nc.vector.tensor_scalar_add(aa_raw[:], aa_ps[:], aab2[:, eb:eb + 1])
                    aa_sq = wp.tile([128, N], F32R, tag="seqsq", name=f"aasq{eb}")
                    nc.scalar.activation(aa_sq[:, :J], aa_raw[:], AF.Square)
                    nsqa_ps = pb512.tile([2, J], F32, tag="b512", name=f"nsqa{eb}")
                    nc.tensor.matmul(nsqa_ps[:], pairmask[:], aa_sq[:, :J], start=True, stop=True)
                    lnn = wp.tile([2, J], F32, tag="lnn", name=f"lnn{eb}")
                    nc.scalar.activation(lnn[:], nsqa_ps[:], AF.Ln, bias=cb[:2, 1:2], scale=1.0)
                    inva = wp.tile([2, J], F32R, tag="inva", name=f"inva{eb}")
                    nc.scalar.activation(inva[:], lnn[:], AF.Exp, bias=0.0, scale=-0.5)
                    nc.vector.tensor_mul(inva[:], inva[:].bitcast(F32), mask2[:])
                    bc_ps = pb512.tile([128, J], F32, tag="b512", name=f"bc{eb}")
                    nc.tensor.matmul(bc_ps[:], sel2[:], inva[:], start=True, stop=True)
                    aa_nrm = wp.tile([128, J], F32R, tag="aanrm", bufs=3, name=f"aanrm{eb}")
                    nc.vector.tensor_mul(aa_nrm[:], aa_raw[:], bc_ps[:])

                    # seq projection for this e-block
                    sq_ps = [pproj.tile([128, NH], F32, tag="proj", name=f"sqps{eb}_{c}") for c in range(2)]
                    for hf in range(2):
                        ws = wp.tile([128, (KS // 2) * 128], F32R, tag="wseq", name=f"ws{eb}_{hf}")
                        nc.gpsimd.dma_start(
                            out=ws[:],
                            in_=seqw_ext.ap().rearrange("(kb p) e -> p kb e", p=128)[:, (KS // 2) * hf:(KS // 2) * (hf + 1), 128 * eb:128 * (eb + 1)])
                        for k in range(KS // 2):
                            kb = hf * (KS // 2) + k
                            for c in range(2):
                                nc.tensor.matmul(sq_ps[c][:], ws[:, 128 * k:128 * (k + 1)], seqT[kb][:, NH * c:NH * (c + 1)],
                                                 start=(kb == 0), stop=(kb == KS - 1))
                    seq_sb = wp.tile([128, N], F32R, tag="seq", bufs=3, name=f"seqsb{eb}")
                    for c in range(2):
                        nc.vector.tensor_scalar_add(seq_sb[:, NH * c:NH * (c + 1)], sq_ps[c][:], seqb2[:, eb:eb + 1])
                    seq_sq = wp.tile([128, N], F32R, tag="seqsq", name=f"seqsq{eb}")
                    nc.scalar.activation(seq_sq[:], seq_sb[:].bitcast(F32), AF.Square)
                    nsq_sb = wp.tile([2, N], F32, tag="nsq", name=f"nsq{eb}")
                    for c in range(2):
                        nsqs_ps = psm2.tile([2, NH], F32, tag="sm2", name=f"nsqs{eb}_{c}")
                        nc.tensor.matmul(nsqs_ps[:], pairmask[:], seq_sq[:, NH * c:NH * (c + 1)], start=True, stop=True)
                        nc.vector.tensor_copy(nsq_sb[:, NH * c:NH * (c + 1)], nsqs_ps[:])
                    for ib in range(IB):
                        tp_ps = psm2.tile([128, 2], F32, tag="sm2", name=f"itp{eb}_{ib}")
                        nc.tensor.transpose(tp_ps[:], nsq_sb[:, 128 * ib:128 * (ib + 1)], eye2[:])
                        lns = wp.tile([128, 2], F32, tag="lns", name=f"lns{eb}_{ib}")
                        nc.scalar.activation(lns[:], tp_ps[:], AF.Ln, bias=cb[:, 1:2], scale=1.0)
                        nc.scalar.activation(invsT[ib][:, 2 * eb:2 * eb + 2], lns[:], AF.Exp, bias=cb[:, 2:3], scale=-0.5)

                    # interactions + exp-accumulate for heads 2eb, 2eb+1
                    for hh in range(2):
                        h = 2 * eb + hh
                        for ib in range(IB):
                            int_ps = pips.tile([128, J], F32, tag="ips", name=f"int{h}_{ib}")
                            nc.tensor.matmul(int_ps[:], seq_sb[64 * hh:64 * (hh + 1), 128 * ib:128 * (ib + 1)],
                                             aa_nrm[64 * hh:64 * (hh + 1), :], start=True, stop=True)
                            nc.scalar.activation(int_ps[:], int_ps[:], AF.Exp,
                                                 bias=cb[:, 0:1], scale=invsT[ib][:, h:h + 1],
                                                 accum_out=s_t[ib][:, h:h + 1])

            if stage == "eb":
                for ib in range(IB):
                    nc.sync.dma_start(out=out_ext[128 * ib:128 * (ib + 1)], in_=s_t[ib][:, 0:1])

            # ---- phase 2: finalize ----
            if stage == "full":
                for ib in range(IB):
                    r1 = wp.tile([128, H], F32, tag="r1", name=f"r1_{ib}")
                    nc.scalar.activation(r1[:], s_t[ib][:], AF.Ln, bias=cb[:, 1:2], scale=1.0)
                    nc.vector.tensor_scalar(r1[:], r1[:], 0.01, cvec[:, 0:1],
                                            op0=AL.mult, op1=AL.add)
                    junk = wp.tile([128, H], F32, tag="junk", name=f"junk{ib}")
                    pp_t = wp.tile([128, 1], F32, tag="pp_t", name=f"pp{ib}")
                    nc.vector.tensor_mul(junk[:], r1[:], WV[:])
                    nc.vector.reduce_sum(pp_t[:], junk[:], axis=mybir.AxisListType.X)
                    nc.vector.tensor_scalar_min(pp_t[:], pp_t[:], 80.0)
                    nc.scalar.activation(pp_t[:], pp_t[:], AF.Exp, bias=cb[:, 3:4], scale=1.0)
                    nc.scalar.activation(pp_t[:], pp_t[:], AF.Ln, bias=1.0, scale=1.0)
                    nc.sync.dma_start(out=out_ext[128 * ib:128 * (ib + 1)], in_=pp_t[:])

    nc.compile()
    _GRAPH_CACHE[key] = nc
    return nc


def _prep_in_maps(inputs):
    seq_embed = np.ascontiguousarray(inputs["seq_embed"], dtype=np.float32)
    aa_embed = np.ascontiguousarray(inputs["aa_embed"], dtype=np.float32)
    ctx = np.ascontiguousarray(inputs["contextual_embed"], dtype=np.float32)
    aa_mask = np.asarray(inputs["aa_mask"])
    seq_w = np.ascontiguousarray(inputs["seq_w"], dtype=np.float32)
    seq_b = np.asarray(inputs["seq_b"], dtype=np.float32)
    aa_w = np.ascontiguousarray(inputs["aa_w"], dtype=np.float32)
    aa_b = np.asarray(inputs["aa_b"], dtype=np.float32)
    tlw = np.asarray(inputs["to_logits_w"], dtype=np.float32)
    ctx_w = np.asarray(inputs["ctx_w"], dtype=np.float32)
    ctx_b = np.asarray(inputs["ctx_b"], dtype=np.float32)
    pred_w = np.ascontiguousarray(inputs["pred_w"], dtype=np.float32)

    # permute gating space from h-major (h*32+e) to e-major (e*32+h)
    perm = (np.arange(H * H).reshape(H, H).T).reshape(-1)  # new[e*32+h] = old[h*32+e]
    ctx_wp = np.ascontiguousarray(ctx_w[:, perm])
    ctx_bp = np.ascontiguousarray(ctx_b[perm])[None, :]
    tlwT = np.ascontiguousarray(tlw.T.reshape(1, H * H))   # [1,(e h)]

    seq_b2 = np.ascontiguousarray(seq_b.reshape(EB, 128).T)
    aa_b2 = np.ascontiguousarray(aa_b.reshape(EB, 128).T)
    eye128 = np.eye(128, dtype=np.float32)
    eye2 = np.eye(2, dtype=np.float32)
    pairmask = np.zeros((128, 2), dtype=np.float32)
    pairmask[:64, 0] = 1.0
    pairmask[64:, 1] = 1.0
    sel2 = np.zeros((2, 128), dtype=np.float32)
    sel2[0, :64] = 1.0
    sel2[1, 64:] = 1.0
    ones1 = np.ones((1, 128), dtype=np.float32)

    in_maps = []
    for b in range(B):
        m = aa_mask[b].astype(np.float32)
        n_b = max(float(m.sum()), 1.0)
        cval = 0.01 * (40.0 - 2.0 * math.log(n_b))  # reference's logavgexp subtracts ln n twice
        in_maps.append({
            "seq": seq_embed[b],
            "aa": aa_embed[b],
            "seq_w": seq_w,
            "aa_w": aa_w,
            "seq_b2": seq_b2,
            "aa_b2": aa_b2,
            "ctxT": np.ascontiguousarray(ctx[b].reshape(CTX_D // 128, 128).T),
            "ctx_wp": ctx_wp,
            "ctx_bp": ctx_bp,
            "tlwT": tlwT,
            "pred_w": pred_w,
            "mask2": np.ascontiguousarray(np.broadcast_to(m, (2, J))),
            "cvec": np.full((128, 1), cval, dtype=np.float32),
            "eye128": eye128,
            "eye2": eye2,
            "pairmask": pairmask,
            "sel2": sel2,
            "ones1": ones1,
        })
    return in_maps


def _run(inputs, trace=False, stage="full", n_cores=B):
    from concourse.bass_utils import run_bass_kernel_spmd
    pred_b_val = float(np.asarray(inputs["pred_b"]).reshape(-1)[0])
    nc = _build(pred_b_val, stage=stage)
    in_maps = _prep_in_maps(inputs)
    res = run_bass_kernel_spmd(nc, in_maps[:n_cores], core_ids=list(range(n_cores)), trace=trace)
    out = np.stack([res.results[c]["out"] for c in range(n_cores)], axis=0)
    return out, res


def kernel(**inputs) -> np.ndarray:
    out, _ = _run(inputs, trace=False)
    return out


# revision 10
# speedup vs baseline: 1.6657x; 1.6657x over previous
"""Trainium2 Bass kernel for the AdapterModel problem.

Data-parallel over batch: core b computes pred[b] = f(seq_embed[b], aa_embed[b], ...).
No collectives needed (B == n_cores == 8); host gathers per-core outputs.

Math per core (N=896 seq positions, J=512 aa positions, H=32 heads, D=64):
  seq_lat^T[e,i] = seq_w^T @ seq_embed^T + seq_b     (e = h*64+d, PE, f32r)
  aa_lat^T[e,j]  = aa_w^T @ aa_embed^T + aa_b
  aa_norm = aa_lat * rsqrt(sum_d aa_lat^2) * mask    (norms via pairmask matmul)
  G[h,i,j] = <seq_lat_h[:,i], aa_norm_h[:,j]>        (K=64 matmuls)
  s[i,h] = sum_j exp(G * (100*rsqrt(nsq_seq))_i - 40)    (ACT exp + accum_out)
  r[i,h] = 0.01*ln(s) + 0.01*(40 - 2*ln n)   (reference's logavgexp subtracts ln n twice)
  gating: wv = (to_logits_w .* sigmoid(ctx @ ctx_w + ctx_b)) @ pred_w
  out[i] = softplus(r[i,:] @ wv + pred_b)

ACT runs ONLY Exp during the hot loop (one table load); rsqrt is a DVE Newton
iteration (bit-trick seed), squares are DVE multiplies, sigmoid/softplus are
built from exp + reciprocal/ln, and the phase-2 Ln/Exp calls are batched by
function to avoid ACT table-set thrash (~2.7us per switch).
"""
import sys

if "/opt/trn_rl_repo" not in sys.path:
    sys.path.insert(0, "/opt/trn_rl_repo")

import math
import numpy as np

H = 32
D = 64
E = H * D            # 2048
SEQ_D = 3072
AA_D = 1280
CTX_D = 768
B, N, J = 8, 896, 512
KS = SEQ_D // 128    # 24
KA = AA_D // 128     # 10
EB = E // 128        # 16
IB = N // 128        # 7
JB = J // 128        # 4
NH = N // 2          # 448
MAGIC = 0x5F3759DF

_GRAPH_CACHE = {}


def _build(pred_b_val: float, stage: str = "full"):
    key = (float(pred_b_val), stage)
    if key in _GRAPH_CACHE:
        return _GRAPH_CACHE[key]

    import concourse.bacc as bacc
    import concourse.mybir as mybir
    import concourse.tile as tile

    F32 = mybir.dt.float32
    F32R = mybir.dt.float32r
    U32 = mybir.dt.uint32
    AF = mybir.ActivationFunctionType
    AL = mybir.AluOpType

    nc = bacc.Bacc("TRN2", target_bir_lowering=False, debug=False, num_devices=8)

    seq_ext = nc.dram_tensor("seq", [N, SEQ_D], F32, kind="ExternalInput")
    aa_ext = nc.dram_tensor("aa", [J, AA_D], F32, kind="ExternalInput")
    seqw_ext = nc.dram_tensor("seq_w", [SEQ_D, E], F32, kind="ExternalInput")
    aaw_ext = nc.dram_tensor("aa_w", [AA_D, E], F32, kind="ExternalInput")
    seqb_ext = nc.dram_tensor("seq_b2", [128, EB], F32, kind="ExternalInput")
    aab_ext = nc.dram_tensor("aa_b2", [128, EB], F32, kind="ExternalInput")
    ctxT_ext = nc.dram_tensor("ctxT", [128, CTX_D // 128], F32, kind="ExternalInput")
    ctxw_ext = nc.dram_tensor("ctx_wp", [CTX_D, H * H], F32, kind="ExternalInput")
    ctxb_ext = nc.dram_tensor("ctx_bp", [1, H * H], F32, kind="ExternalInput")
    tlw_ext = nc.dram_tensor("tlwT", [1, H * H], F32, kind="ExternalInput")
    predw_ext = nc.dram_tensor("pred_w", [H, 1], F32, kind="ExternalInput")
    maskT_ext = nc.dram_tensor("maskT", [128, 2 * JB], F32, kind="ExternalInput")
    cvec_ext = nc.dram_tensor("cvec", [128, 1], F32, kind="ExternalInput")
    eye_ext = nc.dram_tensor("eye128", [128, 128], F32, kind="ExternalInput")
    eye2_ext = nc.dram_tensor("eye2", [2, 2], F32, kind="ExternalInput")
    pm_ext = nc.dram_tensor("pairmask", [128, 2], F32, kind="ExternalInput")
    sel2_ext = nc.dram_tensor("sel2", [2, 128], F32, kind="ExternalInput")
    ones1_ext = nc.dram_tensor("ones1", [1, 128], F32, kind="ExternalInput")
    out_ext = nc.dram_tensor("out", [N], F32, kind="ExternalOutput")

    KCTX = CTX_D // 128  # 6

    with tile.TileContext(nc) as tc:
        with tc.tile_pool(name="persist", bufs=1) as pp, \
             tc.tile_pool(name="work", bufs=2) as wp, \
             tc.tile_pool(name="pproj", bufs=3, space="PSUM") as pproj, \
             tc.tile_pool(name="pb512", bufs=2, space="PSUM") as pb512, \
             tc.tile_pool(name="pips", bufs=2, space="PSUM") as pips, \
             tc.tile_pool(name="psm2", bufs=1, space="PSUM") as psm2:

            def dummy_out(src_ap):
                dummy = pp.tile([128, 1], F32, tag="dummy", name="dummy")
                nc.vector.tensor_copy(dummy[:], src_ap)
                for ib in range(IB):
                    nc.sync.dma_start(out=out_ext[128 * ib:128 * (ib + 1)], in_=dummy[:])

            # ---- constants / tiny inputs ----
            eye = pp.tile([128, 128], F32, tag="eye")
            nc.sync.dma_start(out=eye[:], in_=eye_ext[:])
            eye2 = pp.tile([2, 2], F32, tag="eye2")
            nc.sync.dma_start(out=eye2[:], in_=eye2_ext[:])
            pairmask = pp.tile([128, 2], F32R, tag="pairmask")
            nc.gpsimd.dma_start(out=pairmask[:], in_=pm_ext[:])
            sel2 = pp.tile([2, 128], F32R, tag="sel2")
            nc.gpsimd.dma_start(out=sel2[:], in_=sel2_ext[:])
            ones1 = pp.tile([1, 128], F32R, tag="ones1")
            nc.gpsimd.dma_start(out=ones1[:], in_=ones1_ext[:])
            maskT = pp.tile([128, 2 * JB], F32, tag="maskT")
            nc.sync.dma_start(out=maskT[:], in_=maskT_ext[:])
            cvec = pp.tile([128, 1], F32, tag="cvec")
            nc.sync.dma_start(out=cvec[:], in_=cvec_ext[:])
            seqb2 = pp.tile([128, EB], F32, tag="seqb2")
            nc.sync.dma_start(out=seqb2[:], in_=seqb_ext[:])
            aab2 = pp.tile([128, EB], F32, tag="aab2")
            nc.sync.dma_start(out=aab2[:], in_=aab_ext[:])
            # const bias columns: [0]=-40, [1]=1e-30, [2]=pred_b
            cb = pp.tile([128, 3], F32, tag="cb")
            nc.gpsimd.memset(cb[:, 0:1], -40.0)
            nc.gpsimd.memset(cb[:, 1:2], 1e-30)
            nc.gpsimd.memset(cb[:, 2:3], float(pred_b_val))
            magic = pp.tile([128, 16], U32, tag="magic")
            nc.gpsimd.memset(magic[:], MAGIC)

            def rsqrt_newton(name, xin_ap, w, scale_mul):
                """SBUF f32 [128, w] <- rsqrt(xin * scale_mul); xin may be PSUM."""
                x = wp.tile([128, 16], F32, tag="nwx", name=f"nwx{name}")
                nc.vector.tensor_scalar(x[:, :w], xin_ap, scale_mul, 1e-35, op0=AL.mult, op1=AL.add)
                u = wp.tile([128, 16], U32, tag="nwu", name=f"nwu{name}")
                nc.vector.tensor_scalar(u[:, :w], x[:, :w].bitcast(U32), 1, None, op0=AL.logical_shift_right)
                y0u = wp.tile([128, 16], U32, tag="nwy0", name=f"nwy0{name}")
                nc.vector.tensor_tensor(out=y0u[:, :w], in0=magic[:, :w], in1=u[:, :w], op=AL.subtract)
                y = wp.tile([128, 16], F32, tag="nwy", name=f"nwy{name}")
                t = wp.tile([128, 16], F32, tag="nwt", name=f"nwt{name}")
                nc.vector.tensor_mul(t[:, :w], y0u[:, :w].bitcast(F32), y0u[:, :w].bitcast(F32))
                nc.vector.tensor_mul(t[:, :w], t[:, :w], x[:, :w])
                nc.vector.tensor_scalar(t[:, :w], t[:, :w], -0.5, 1.5, op0=AL.mult, op1=AL.add)
                nc.vector.tensor_mul(y[:, :w], y0u[:, :w].bitcast(F32), t[:, :w])
                nc.vector.tensor_mul(t[:, :w], y[:, :w], y[:, :w])
                nc.vector.tensor_mul(t[:, :w], t[:, :w], x[:, :w])
                nc.vector.tensor_scalar(t[:, :w], t[:, :w], -0.5, 1.5, op0=AL.mult, op1=AL.add)
                nc.vector.tensor_mul(y[:, :w], y[:, :w], t[:, :w])
                return y

            # ---- gating chain ----
            ctxT = pp.tile([128, KCTX], F32R, tag="ctxT")
            nc.gpsimd.dma_start(out=ctxT[:], in_=ctxT_ext[:])
            g_ps = [pips.tile([1, 512], F32, tag="ips", name=f"gps{i}") for i in range(2)]
            for c in range(KCTX):
                wctx = wp.tile([128, H * H], F32R, tag="wseq", name=f"wctx{c}")
                nc.gpsimd.dma_start(out=wctx[:], in_=ctxw_ext[128 * c:128 * (c + 1), :])
                for half in range(2):
                    nc.tensor.matmul(g_ps[half][:], ctxT[:, c:c + 1], wctx[:, 512 * half:512 * (half + 1)],
                                     start=(c == 0), stop=(c == KCTX - 1))
            ctxb = pp.tile([1, H * H], F32, tag="ctxb")
            nc.sync.dma_start(out=ctxb[:], in_=ctxb_ext[:])
            g_sb = pp.tile([1, H * H], F32, tag="g_sb")
            for half in range(2):
                nc.vector.tensor_add(g_sb[:, 512 * half:512 * (half + 1)], g_ps[half][:], ctxb[:, 512 * half:512 * (half + 1)])
            # sigmoid(x) = 1/(1+exp(-x))
            sig = pp.tile([1, H * H], F32, tag="sig")
            nc.scalar.activation(sig[:], g_sb[:], AF.Exp, bias=0.0, scale=-1.0)
            nc.vector.tensor_scalar_add(sig[:], sig[:], 1.0)
            nc.vector.reciprocal(sig[:], sig[:])
            tlw = pp.tile([1, H * H], F32, tag="tlw")
            nc.sync.dma_start(out=tlw[:], in_=tlw_ext[:])
            nc.vector.tensor_mul(sig[:], sig[:], tlw[:])   # w_b^T flat, e-major
            wb_dram = nc.dram_tensor("wb_bounce", [H, H], F32)
            nc.sync.dma_start(out=wb_dram.ap().rearrange("e h -> (e h)")[None, :], in_=sig[:])
            wbT = pp.tile([H, H], F32R, tag="wbT")
            nc.gpsimd.dma_start(out=wbT[:], in_=wb_dram[:])
            predw = pp.tile([H, 1], F32R, tag="predw")
            nc.gpsimd.dma_start(out=predw[:], in_=predw_ext[:])
            wv_ps = pips.tile([1, H], F32, tag="ips")
            nc.tensor.matmul(wv_ps[:], predw[:], wbT[:], start=True, stop=True)
            wv_sb = pp.tile([1, H], F32R, tag="wv_sb")
            nc.vector.tensor_copy(wv_sb[:], wv_ps[:])
            WV_ps = pips.tile([128, H], F32, tag="ips")
            nc.tensor.matmul(WV_ps[:], ones1[:], wv_sb[:], start=True, stop=True)
            WV = pp.tile([128, H], F32, tag="WV")
            nc.vector.tensor_copy(WV[:], WV_ps[:])

            if stage == "gating":
                dummy_out(WV[:, 0:1])

            # ---- phase 0: transpose aa_embed and seq_embed ----
            if stage in ("tpose", "eb", "full"):
                aaT = [pp.tile([128, J], F32R, tag=f"aat{kb}", name=f"aat{kb}") for kb in range(KA)]
                for jb in range(JB):
                    nat = wp.tile([128, AA_D], F32, tag="nat", bufs=2, name=f"anat{jb}")
                    nc.sync.dma_start(out=nat[:], in_=aa_ext[128 * jb:128 * (jb + 1), :])
                    for kb in range(KA):
                        tp = pips.tile([128, 128], F32, tag="ips", name=f"atp{jb}_{kb}")
                        nc.tensor.transpose(tp[:], nat[:, 128 * kb:128 * (kb + 1)], eye[:])
                        nc.any.tensor_copy(aaT[kb][:, 128 * jb:128 * (jb + 1)], tp[:])

                seqT = [pp.tile([128, N], F32R, tag=f"ast{kb}", name=f"ast{kb}") for kb in range(KS)]
                for ib in range(IB):
                    for hf in range(2):
                        nat = wp.tile([128, SEQ_D // 2], F32, tag="nat", bufs=2, name=f"snat{ib}_{hf}")
                        nc.sync.dma_start(out=nat[:], in_=seq_ext[128 * ib:128 * (ib + 1), (SEQ_D // 2) * hf:(SEQ_D // 2) * (hf + 1)])
                        for k in range(KS // 2):
                            kb = hf * (KS // 2) + k
                            tp = pips.tile([128, 128], F32, tag="ips", name=f"stp{ib}_{kb}")
                            nc.tensor.transpose(tp[:], nat[:, 128 * k:128 * (k + 1)], eye[:])
                            nc.any.tensor_copy(seqT[kb][:, 128 * ib:128 * (ib + 1)], tp[:])

            if stage == "tpose":
                dummy_out(seqT[0][:, 0:1].bitcast(F32))

            # ---- phase 1: per head-pair block ----
            if stage in ("eb", "full"):
                s_t = [pp.tile([128, H], F32, tag=f"s_t{ib}", name=f"s_t{ib}") for ib in range(IB)]

                for eb in range(EB):
                    # aa projection for this e-block
                    aa_ps = pproj.tile([128, J], F32, tag="proj", name=f"aaps{eb}")
                    waa = wp.tile([128, KA * 128], F32R, tag="waa", name=f"waa{eb}")
                    nc.gpsimd.dma_start(
                        out=waa[:],
                        in_=aaw_ext.ap().rearrange("(kb p) e -> p kb e", p=128)[:, :, 128 * eb:128 * (eb + 1)])
                    for kb in range(KA):
                        nc.tensor.matmul(aa_ps[:], waa[:, 128 * kb:128 * (kb + 1)], aaT[kb][:],
                                         start=(kb == 0), stop=(kb == KA - 1))
                    aa_raw = wp.tile([128, J], F32, tag="araw", name=f"araw{eb}")
                    nc.vector.tensor_scalar_add(aa_raw[:], aa_ps[:], aab2[:, eb:eb + 1])
                    aa_sq = wp.tile([128, N], F32R, tag="seqsq", name=f"aasq{eb}")
                    nc.vector.tensor_mul(aa_sq[:, :J], aa_raw[:], aa_raw[:])
                    nsqa_ps = pb512.tile([2, J], F32, tag="b512", name=f"nsqa{eb}")
                    nc.tensor.matmul(nsqa_ps[:], pairmask[:], aa_sq[:, :J], start=True, stop=True)
                    nsqa_sb = wp.tile([2, J], F32, tag="nsq", bufs=3, name=f"nsqa_sb{eb}")
                    nc.vector.tensor_copy(nsqa_sb[:], nsqa_ps[:])
                    ibat_a = psm2.tile([128, 2 * JB], F32, tag="sm2", name=f"ibata{eb}")
                    for c in range(JB):
                        nc.tensor.transpose(ibat_a[:, 2 * c:2 * c + 2], nsqa_sb[:, 128 * c:128 * (c + 1)], eye2[:])
                    ya = rsqrt_newton(f"a{eb}", ibat_a[:, :2 * JB], 2 * JB, 1.0)
                    nc.vector.tensor_mul(ya[:, :2 * JB], ya[:, :2 * JB], maskT[:])
                    rowa_ps = pb512.tile([2, J], F32, tag="b512", name=f"rowa{eb}")
                    for c in range(JB):
                        nc.tensor.transpose(rowa_ps[:, 128 * c:128 * (c + 1)], ya[:, 2 * c:2 * c + 2], eye[:])
                    inva = wp.tile([2, J], F32R, tag="inva", name=f"inva{eb}")
                    nc.vector.tensor_copy(inva[:], rowa_ps[:])
                    bc_ps = pb512.tile([128, J], F32, tag="b512", name=f"bc{eb}")
                    nc.tensor.matmul(bc_ps[:], sel2[:], inva[:], start=True, stop=True)
                    aa_nrm = wp.tile([128, J], F32R, tag="aanrm", bufs=3, name=f"aanrm{eb}")
                    nc.vector.tensor_mul(aa_nrm[:], aa_raw[:], bc_ps[:])

                    # seq projection for this e-block
                    sq_ps = [pproj.tile([128, NH], F32, tag="proj", name=f"sqps{eb}_{c}") for c in range(2)]
                    for hf in range(2):
                        ws = wp.tile([128, (KS // 2) * 128], F32R, tag="wseq", name=f"ws{eb}_{hf}")
                        nc.gpsimd.dma_start(
                            out=ws[:],
                            in_=seqw_ext.ap().rearrange("(kb p) e -> p kb e", p=128)[:, (KS // 2) * hf:(KS // 2) * (hf + 1), 128 * eb:128 * (eb + 1)])
                        for k in range(KS // 2):
                            kb = hf * (KS // 2) + k
                            for c in range(2):
                                nc.tensor.matmul(sq_ps[c][:], ws[:, 128 * k:128 * (k + 1)], seqT[kb][:, NH * c:NH * (c + 1)],
                                                 start=(kb == 0), stop=(kb == KS - 1))
                    seq_sb = wp.tile([128, N], F32R, tag="seq", bufs=3, name=f"seqsb{eb}")
                    for c in range(2):
                        nc.vector.tensor_scalar_add(seq_sb[:, NH * c:NH * (c + 1)], sq_ps[c][:], seqb2[:, eb:eb + 1])
                    seq_sq = wp.tile([128, N], F32R, tag="seqsq", name=f"seqsq{eb}")
                    nc.vector.tensor_mul(seq_sq[:], seq_sb[:].bitcast(F32), seq_sb[:].bitcast(F32))
                    nsq_sb = wp.tile([2, N], F32, tag="nsq", bufs=3, name=f"nsq{eb}")
                    for c in range(2):
                        nsqs_ps = psm2.tile([2, NH], F32, tag="sm2", name=f"nsqs{eb}_{c}")
                        nc.tensor.matmul(nsqs_ps[:], pairmask[:], seq_sq[:, NH * c:NH * (c + 1)], start=True, stop=True)
                        nc.vector.tensor_copy(nsq_sb[:, NH * c:NH * (c + 1)], nsqs_ps[:])
                    ibat_s = psm2.tile([128, 2 * IB], F32, tag="sm2", name=f"ibats{eb}")
                    for ib in range(IB):
                        nc.tensor.transpose(ibat_s[:, 2 * ib:2 * ib + 2], nsq_sb[:, 128 * ib:128 * (ib + 1)], eye2[:])
                    invsT = rsqrt_newton(f"s{eb}", ibat_s[:, :2 * IB], 2 * IB, 1e-4)

                    # interactions + exp-accumulate for heads 2eb, 2eb+1
                    for hh in range(2):
                        h = 2 * eb + hh
                        for ib in range(IB):
                            int_ps = pips.tile([128, J], F32, tag="ips", name=f"int{h}_{ib}")
                            nc.tensor.matmul(int_ps[:], seq_sb[64 * hh:64 * (hh + 1), 128 * ib:128 * (ib + 1)],
                                             aa_nrm[64 * hh:64 * (hh + 1), :], start=True, stop=True)
                            nc.scalar.activation(int_ps[:], int_ps[:], AF.Exp,
                                                 bias=cb[:, 0:1], scale=invsT[:, 2 * ib + hh:2 * ib + hh + 1],
                                                 accum_out=s_t[ib][:, h:h + 1])

            if stage == "eb":
                for ib in range(IB):
                    nc.sync.dma_start(out=out_ext[128 * ib:128 * (ib + 1)], in_=s_t[ib][:, 0:1])

            # ---- phase 2: finalize (batched per ACT function) ----
            if stage == "full":
                r1s = [wp.tile([128, H], F32, tag="r1", bufs=IB, name=f"r1_{ib}") for ib in range(IB)]
                pps = [wp.tile([128, 1], F32, tag="pp_t", bufs=IB, name=f"pp{ib}") for ib in range(IB)]
                for ib in range(IB):
                    nc.scalar.activation(r1s[ib][:], s_t[ib][:], AF.Ln, bias=cb[:, 1:2], scale=1.0)
                junk = wp.tile([128, H], F32, tag="junk", bufs=1, name="junk")
                for ib in range(IB):
                    nc.vector.tensor_scalar(r1s[ib][:], r1s[ib][:], 0.01, cvec[:, 0:1],
                                            op0=AL.mult, op1=AL.add)
                    nc.vector.tensor_mul(junk[:], r1s[ib][:], WV[:])
                    nc.vector.reduce_sum(pps[ib][:], junk[:], axis=mybir.AxisListType.X)
                    nc.vector.tensor_scalar_min(pps[ib][:], pps[ib][:], 80.0)
                for ib in range(IB):
                    nc.scalar.activation(pps[ib][:], pps[ib][:], AF.Exp, bias=cb[:, 2:3], scale=1.0)
                for ib in range(IB):
                    nc.scalar.activation(pps[ib][:], pps[ib][:], AF.Ln, bias=1.0, scale=1.0)
                    nc.sync.dma_start(out=out_ext[128 * ib:128 * (ib + 1)], in_=pps[ib][:])

    nc.compile()
    _GRAPH_CACHE[key] = nc
    return nc


def _prep_in_maps(inputs):
    seq_embed = np.ascontiguousarray(inputs["seq_embed"], dtype=np.float32)
    aa_embed = np.ascontiguousarray(inputs["aa_embed"], dtype=np.float32)
    ctx = np.ascontiguousarray(inputs["contextual_embed"], dtype=np.float32)
    aa_mask = np.asarray(inputs["aa_mask"])
    seq_w = np.ascontiguousarray(inputs["seq_w"], dtype=np.float32)
    seq_b = np.asarray(inputs["seq_b"], dtype=np.float32)
    aa_w = np.ascontiguousarray(inputs["aa_w"], dtype=np.float32)
    aa_b = np.asarray(inputs["aa_b"], dtype=np.float32)
    tlw = np.asarray(inputs["to_logits_w"], dtype=np.float32)
    ctx_w = np.asarray(inputs["ctx_w"], dtype=np.float32)
    ctx_b = np.asarray(inputs["ctx_b"], dtype=np.float32)
    pred_w = np.ascontiguousarray(inputs["pred_w"], dtype=np.float32)

    # permute gating space from h-major (h*32+e) to e-major (e*32+h)
    perm = (np.arange(H * H).reshape(H, H).T).reshape(-1)  # new[e*32+h] = old[h*32+e]
    ctx_wp = np.ascontiguousarray(ctx_w[:, perm])
    ctx_bp = np.ascontiguousarray(ctx_b[perm])[None, :]
    tlwT = np.ascontiguousarray(tlw.T.reshape(1, H * H))   # [1,(e h)]

    seq_b2 = np.ascontiguousarray(seq_b.reshape(EB, 128).T)
    aa_b2 = np.ascontiguousarray(aa_b.reshape(EB, 128).T)
    eye128 = np.eye(128, dtype=np.float32)
    eye2 = np.eye(2, dtype=np.float32)
    pairmask = np.zeros((128, 2), dtype=np.float32)
    pairmask[:64, 0] = 1.0
    pairmask[64:, 1] = 1.0
    sel2 = np.zeros((2, 128), dtype=np.float32)
    sel2[0, :64] = 1.0
    sel2[1, 64:] = 1.0
    ones1 = np.ones((1, 128), dtype=np.float32)

    in_maps = []
    for b in range(B):
        m = aa_mask[b].astype(np.float32)
        n_b = max(float(m.sum()), 1.0)
        cval = 0.01 * (40.0 - 2.0 * math.log(n_b))  # reference's logavgexp subtracts ln n twice
        mT = np.zeros((128, 2 * JB), dtype=np.float32)
        for c in range(JB):
            mT[:, 2 * c] = m[128 * c:128 * (c + 1)]
            mT[:, 2 * c + 1] = m[128 * c:128 * (c + 1)]
        in_maps.append({
            "seq": seq_embed[b],
            "aa": aa_embed[b],
            "seq_w": seq_w,
            "aa_w": aa_w,
            "seq_b2": seq_b2,
            "aa_b2": aa_b2,
            "ctxT": np.ascontiguousarray(ctx[b].reshape(CTX_D // 128, 128).T),
            "ctx_wp": ctx_wp,
            "ctx_bp": ctx_bp,
            "tlwT": tlwT,
            "pred_w": pred_w,
            "maskT": mT,
            "cvec": np.full((128, 1), cval, dtype=np.float32),
            "eye128": eye128,
            "eye2": eye2,
            "pairmask": pairmask,
            "sel2": sel2,
            "ones1": ones1,
        })
    return in_maps


def _run(inputs, trace=False, stage="full", n_cores=B):
    from concourse.bass_utils import run_bass_kernel_spmd
    pred_b_val = float(np.asarray(inputs["pred_b"]).reshape(-1)[0])
    nc = _build(pred_b_val, stage=stage)
    in_maps = _prep_in_maps(inputs)
    res = run_bass_kernel_spmd(nc, in_maps[:n_cores], core_ids=list(range(n_cores)), trace=trace)
    out = np.stack([res.results[c]["out"] for c in range(n_cores)], axis=0)
    return out, res


def kernel(**inputs) -> np.ndarray:
    out, _ = _run(inputs, trace=False)
    return out


# revision 11
# speedup vs baseline: 1.7352x; 1.0418x over previous
"""Trainium2 Bass kernel for the AdapterModel problem.

Data-parallel over batch: core b computes pred[b] = f(seq_embed[b], aa_embed[b], ...).
No collectives needed (B == n_cores == 8); host gathers per-core outputs.

Math per core (N=896 seq positions, J=512 aa positions, H=32 heads, D=64):
  seq_lat^T[e,i] = seq_w^T @ seq_embed^T + seq_b     (e = h*64+d, PE, f32r)
  aa_lat^T[e,j]  = aa_w^T @ aa_embed^T + aa_b
  aa_norm = aa_lat * rsqrt(sum_d aa_lat^2) * mask    (norms via pairmask matmul)
  G[h,i,j] = <seq_lat_h[:,i], aa_norm_h[:,j]>        (K=64 matmuls)
  s[i,h] = sum_j exp(G * (100*rsqrt(nsq_seq))_i - 40)    (ACT exp + accum_out)
  r[i,h] = 0.01*ln(s) + 0.01*(40 - 2*ln n)   (reference's logavgexp subtracts ln n twice)
  gating: wv = (to_logits_w .* sigmoid(ctx @ ctx_w + ctx_b)) @ pred_w
  out[i] = softplus(r[i,:] @ wv + pred_b)

ACT runs ONLY Exp during the hot loop (one table load); rsqrt is a DVE Newton
iteration (bit-trick seed), squares are DVE multiplies, sigmoid/softplus are
built from exp + reciprocal/ln, and the phase-2 Ln/Exp calls are batched by
function to avoid ACT table-set thrash (~2.7us per switch).
"""
import sys

if "/opt/trn_rl_repo" not in sys.path:
    sys.path.insert(0, "/opt/trn_rl_repo")

import math
import numpy as np

H = 32
D = 64
E = H * D            # 2048
SEQ_D = 3072
AA_D = 1280
CTX_D = 768
B, N, J = 8, 896, 512
KS = SEQ_D // 128    # 24
KA = AA_D // 128     # 10
EB = E // 128        # 16
IB = N // 128        # 7
JB = J // 128        # 4
NH = N // 2          # 448
MAGIC = 0x5F3759DF

_GRAPH_CACHE = {}


def _build(pred_b_val: float, stage: str = "full"):
    key = (float(pred_b_val), stage)
    if key in _GRAPH_CACHE:
        return _GRAPH_CACHE[key]

    import concourse.bacc as bacc
    import concourse.mybir as mybir
    import concourse.tile as tile

    F32 = mybir.dt.float32
    F32R = mybir.dt.float32r
    U32 = mybir.dt.uint32
    AF = mybir.ActivationFunctionType
    AL = mybir.AluOpType

    nc = bacc.Bacc("TRN2", target_bir_lowering=False, debug=False, num_devices=8)

    seq_ext = nc.dram_tensor("seq", [N, SEQ_D], F32, kind="ExternalInput")
    aa_ext = nc.dram_tensor("aa", [J, AA_D], F32, kind="ExternalInput")
    seqw_ext = nc.dram_tensor("seq_w", [SEQ_D, E], F32, kind="ExternalInput")
    aaw_ext = nc.dram_tensor("aa_w", [AA_D, E], F32, kind="ExternalInput")
    seqb_ext = nc.dram_tensor("seq_b2", [128, EB], F32, kind="ExternalInput")
    aab_ext = nc.dram_tensor("aa_b2", [128, EB], F32, kind="ExternalInput")
    ctxT_ext = nc.dram_tensor("ctxT", [128, CTX_D // 128], F32, kind="ExternalInput")
    ctxw_ext = nc.dram_tensor("ctx_wp", [CTX_D, H * H], F32, kind="ExternalInput")
    ctxb_ext = nc.dram_tensor("ctx_bp", [1, H * H], F32, kind="ExternalInput")
    tlw_ext = nc.dram_tensor("tlwT", [1, H * H], F32, kind="ExternalInput")
    predw_ext = nc.dram_tensor("pred_w", [H, 1], F32, kind="ExternalInput")
    maskT_ext = nc.dram_tensor("maskT", [128, 2 * JB], F32, kind="ExternalInput")
    cvec_ext = nc.dram_tensor("cvec", [128, 1], F32, kind="ExternalInput")
    eye_ext = nc.dram_tensor("eye128", [128, 128], F32, kind="ExternalInput")
    eye2_ext = nc.dram_tensor("eye2", [2, 2], F32, kind="ExternalInput")
    pm_ext = nc.dram_tensor("pairmask", [128, 2], F32, kind="ExternalInput")
    sel2_ext = nc.dram_tensor("sel2", [2, 128], F32, kind="ExternalInput")
    ones1_ext = nc.dram_tensor("ones1", [1, 128], F32, kind="ExternalInput")
    out_ext = nc.dram_tensor("out", [N], F32, kind="ExternalOutput")

    KCTX = CTX_D // 128  # 6

    with tile.TileContext(nc) as tc:
        with tc.tile_pool(name="persist", bufs=1) as pp, \
             tc.tile_pool(name="work", bufs=2) as wp, \
             tc.tile_pool(name="pproj", bufs=2, space="PSUM") as pproj, \
             tc.tile_pool(name="pb512", bufs=2, space="PSUM") as pb512, \
             tc.tile_pool(name="pips", bufs=3, space="PSUM") as pips, \
             tc.tile_pool(name="psm2", bufs=1, space="PSUM") as psm2:

            def dummy_out(src_ap):
                dummy = pp.tile([128, 1], F32, tag="dummy", name="dummy")
                nc.vector.tensor_copy(dummy[:], src_ap)
                for ib in range(IB):
                    nc.sync.dma_start(out=out_ext[128 * ib:128 * (ib + 1)], in_=dummy[:])

            # ---- constants / tiny inputs ----
            eye = pp.tile([128, 128], F32, tag="eye")
            nc.sync.dma_start(out=eye[:], in_=eye_ext[:])
            eyer = pp.tile([128, 128], F32R, tag="eyer")
            nc.gpsimd.dma_start(out=eyer[:], in_=eye_ext[:])
            eye2 = pp.tile([2, 2], F32, tag="eye2")
            nc.sync.dma_start(out=eye2[:], in_=eye2_ext[:])
            pairmask = pp.tile([128, 2], F32R, tag="pairmask")
            nc.gpsimd.dma_start(out=pairmask[:], in_=pm_ext[:])
            sel2 = pp.tile([2, 128], F32R, tag="sel2")
            nc.gpsimd.dma_start(out=sel2[:], in_=sel2_ext[:])
            ones1 = pp.tile([1, 128], F32R, tag="ones1")
            nc.gpsimd.dma_start(out=ones1[:], in_=ones1_ext[:])
            maskT = pp.tile([128, 2 * JB], F32, tag="maskT")
            nc.sync.dma_start(out=maskT[:], in_=maskT_ext[:])
            cvec = pp.tile([128, 1], F32, tag="cvec")
            nc.sync.dma_start(out=cvec[:], in_=cvec_ext[:])
            seqb2 = pp.tile([128, EB], F32, tag="seqb2")
            nc.sync.dma_start(out=seqb2[:], in_=seqb_ext[:])
            aab2 = pp.tile([128, EB], F32, tag="aab2")
            nc.sync.dma_start(out=aab2[:], in_=aab_ext[:])
            # const bias columns: [0]=-40, [1]=1e-30, [2]=pred_b
            cb = pp.tile([128, 3], F32, tag="cb")
            nc.gpsimd.memset(cb[:, 0:1], -40.0)
            nc.gpsimd.memset(cb[:, 1:2], 1e-30)
            nc.gpsimd.memset(cb[:, 2:3], float(pred_b_val))
            magic = pp.tile([128, 16], U32, tag="magic")
            nc.gpsimd.memset(magic[:], MAGIC)

            def rsqrt_newton(name, xin_ap, w, scale_mul):
                """SBUF f32 [128, w] <- rsqrt(xin * scale_mul); xin may be PSUM."""
                x = wp.tile([128, 16], F32, tag="nwx", name=f"nwx{name}")
                nc.vector.tensor_scalar(x[:, :w], xin_ap, scale_mul, 1e-35, op0=AL.mult, op1=AL.add)
                u = wp.tile([128, 16], U32, tag="nwu", name=f"nwu{name}")
                nc.vector.tensor_scalar(u[:, :w], x[:, :w].bitcast(U32), 1, None, op0=AL.logical_shift_right)
                y0u = wp.tile([128, 16], U32, tag="nwy0", name=f"nwy0{name}")
                nc.vector.tensor_tensor(out=y0u[:, :w], in0=magic[:, :w], in1=u[:, :w], op=AL.subtract)
                y = wp.tile([128, 16], F32, tag="nwy", name=f"nwy{name}")
                t = wp.tile([128, 16], F32, tag="nwt", name=f"nwt{name}")
                nc.vector.tensor_mul(t[:, :w], y0u[:, :w].bitcast(F32), y0u[:, :w].bitcast(F32))
                nc.vector.tensor_mul(t[:, :w], t[:, :w], x[:, :w])
                nc.vector.tensor_scalar(t[:, :w], t[:, :w], -0.5, 1.5, op0=AL.mult, op1=AL.add)
                nc.vector.tensor_mul(y[:, :w], y0u[:, :w].bitcast(F32), t[:, :w])
                nc.vector.tensor_mul(t[:, :w], y[:, :w], y[:, :w])
                nc.vector.tensor_mul(t[:, :w], t[:, :w], x[:, :w])
                nc.vector.tensor_scalar(t[:, :w], t[:, :w], -0.5, 1.5, op0=AL.mult, op1=AL.add)
                nc.vector.tensor_mul(y[:, :w], y[:, :w], t[:, :w])
                return y

            # ---- gating chain ----
            ctxT = pp.tile([128, KCTX], F32R, tag="ctxT")
            nc.gpsimd.dma_start(out=ctxT[:], in_=ctxT_ext[:])
            g_ps = [pips.tile([1, 512], F32, tag="ips", name=f"gps{i}") for i in range(2)]
            for c in range(KCTX):
                wctx = wp.tile([128, H * H], F32R, tag="wseq", name=f"wctx{c}")
                nc.gpsimd.dma_start(out=wctx[:], in_=ctxw_ext[128 * c:128 * (c + 1), :])
                for half in range(2):
                    nc.tensor.matmul(g_ps[half][:], ctxT[:, c:c + 1], wctx[:, 512 * half:512 * (half + 1)],
                                     start=(c == 0), stop=(c == KCTX - 1))
            ctxb = pp.tile([1, H * H], F32, tag="ctxb")
            nc.sync.dma_start(out=ctxb[:], in_=ctxb_ext[:])
            g_sb = pp.tile([1, H * H], F32, tag="g_sb")
            for half in range(2):
                nc.vector.tensor_add(g_sb[:, 512 * half:512 * (half + 1)], g_ps[half][:], ctxb[:, 512 * half:512 * (half + 1)])
            # sigmoid(x) = 1/(1+exp(-x))
            sig = pp.tile([1, H * H], F32, tag="sig")
            nc.scalar.activation(sig[:], g_sb[:], AF.Exp, bias=0.0, scale=-1.0)
            nc.vector.tensor_scalar_add(sig[:], sig[:], 1.0)
            nc.vector.reciprocal(sig[:], sig[:])
            tlw = pp.tile([1, H * H], F32, tag="tlw")
            nc.sync.dma_start(out=tlw[:], in_=tlw_ext[:])
            nc.vector.tensor_mul(sig[:], sig[:], tlw[:])   # w_b^T flat, e-major
            wb_dram = nc.dram_tensor("wb_bounce", [H, H], F32)
            nc.sync.dma_start(out=wb_dram.ap().rearrange("e h -> (e h)")[None, :], in_=sig[:])
            wbT = pp.tile([H, H], F32R, tag="wbT")
            nc.gpsimd.dma_start(out=wbT[:], in_=wb_dram[:])
            predw = pp.tile([H, 1], F32R, tag="predw")
            nc.gpsimd.dma_start(out=predw[:], in_=predw_ext[:])
            wv_ps = pips.tile([1, H], F32, tag="ips")
            nc.tensor.matmul(wv_ps[:], predw[:], wbT[:], start=True, stop=True)
            wv_sb = pp.tile([1, H], F32R, tag="wv_sb")
            nc.vector.tensor_copy(wv_sb[:], wv_ps[:])
            WV_ps = pips.tile([128, H], F32, tag="ips")
            nc.tensor.matmul(WV_ps[:], ones1[:], wv_sb[:], start=True, stop=True)
            WV = pp.tile([128, H], F32, tag="WV")
            nc.vector.tensor_copy(WV[:], WV_ps[:])

            if stage == "gating":
                dummy_out(WV[:, 0:1])

            # ---- phase 0: transpose aa_embed and seq_embed ----
            if stage in ("tpose", "eb", "full"):
                aaT = [pp.tile([128, J], F32R, tag=f"aat{kb}", name=f"aat{kb}") for kb in range(KA)]
                for jb in range(JB):
                    nat = wp.tile([128, AA_D], F32R, tag="nat", bufs=2, name=f"anat{jb}")
                    nc.gpsimd.dma_start(out=nat[:], in_=aa_ext[128 * jb:128 * (jb + 1), :])
                    for kb in range(KA):
                        tp = pips.tile([128, 128], F32R, tag="ips", name=f"atp{jb}_{kb}")
                        nc.tensor.transpose(tp[:], nat[:, 128 * kb:128 * (kb + 1)], eyer[:])
                        nc.vector.tensor_copy(aaT[kb][:, 128 * jb:128 * (jb + 1)], tp[:])

                seqT = [pp.tile([128, N], F32R, tag=f"ast{kb}", name=f"ast{kb}") for kb in range(KS)]
                for ib in range(IB):
                    for hf in range(2):
                        nat = wp.tile([128, SEQ_D // 2], F32R, tag="nat", bufs=2, name=f"snat{ib}_{hf}")
                        nc.gpsimd.dma_start(out=nat[:], in_=seq_ext[128 * ib:128 * (ib + 1), (SEQ_D // 2) * hf:(SEQ_D // 2) * (hf + 1)])
                        for k in range(KS // 2):
                            kb = hf * (KS // 2) + k
                            tp = pips.tile([128, 128], F32R, tag="ips", name=f"stp{ib}_{kb}")
                            nc.tensor.transpose(tp[:], nat[:, 128 * k:128 * (k + 1)], eyer[:])
                            nc.vector.tensor_copy(seqT[kb][:, 128 * ib:128 * (ib + 1)], tp[:])

            if stage == "tpose":
                dummy_out(seqT[0][:, 0:1].bitcast(F32))

            # ---- phase 1: per head-pair block ----
            if stage in ("eb", "full"):
                s_t = [pp.tile([128, H], F32, tag=f"s_t{ib}", name=f"s_t{ib}") for ib in range(IB)]

                for eb in range(EB):
                    # aa projection for this e-block
                    aa_ps = pproj.tile([128, J], F32, tag="proj", name=f"aaps{eb}")
                    waa = wp.tile([128, KA * 128], F32R, tag="waa", name=f"waa{eb}")
                    nc.gpsimd.dma_start(
                        out=waa[:],
                        in_=aaw_ext.ap().rearrange("(kb p) e -> p kb e", p=128)[:, :, 128 * eb:128 * (eb + 1)])
                    for kb in range(KA):
                        nc.tensor.matmul(aa_ps[:], waa[:, 128 * kb:128 * (kb + 1)], aaT[kb][:],
                                         start=(kb == 0), stop=(kb == KA - 1))
                    aa_raw = wp.tile([128, J], F32, tag="araw", name=f"araw{eb}")
                    nc.vector.tensor_scalar_add(aa_raw[:], aa_ps[:], aab2[:, eb:eb + 1])
                    aa_sq = wp.tile([128, N], F32R, tag="seqsq", name=f"aasq{eb}")
                    nc.vector.tensor_mul(aa_sq[:, :J], aa_raw[:], aa_raw[:])
                    nsqa_ps = pb512.tile([2, J], F32, tag="b512", name=f"nsqa{eb}")
                    nc.tensor.matmul(nsqa_ps[:], pairmask[:], aa_sq[:, :J], start=True, stop=True)
                    nsqa_sb = wp.tile([2, J], F32, tag="nsq", bufs=3, name=f"nsqa_sb{eb}")
                    nc.vector.tensor_copy(nsqa_sb[:], nsqa_ps[:])
                    ibat_a = psm2.tile([128, 2 * JB], F32, tag="sm2", name=f"ibata{eb}")
                    for c in range(JB):
                        nc.tensor.transpose(ibat_a[:, 2 * c:2 * c + 2], nsqa_sb[:, 128 * c:128 * (c + 1)], eye2[:])
                    ya = rsqrt_newton(f"a{eb}", ibat_a[:, :2 * JB], 2 * JB, 1.0)
                    nc.vector.tensor_mul(ya[:, :2 * JB], ya[:, :2 * JB], maskT[:])
                    rowa_ps = pb512.tile([2, J], F32, tag="b512", name=f"rowa{eb}")
                    for c in range(JB):
                        nc.tensor.transpose(rowa_ps[:, 128 * c:128 * (c + 1)], ya[:, 2 * c:2 * c + 2], eye[:])
                    inva = wp.tile([2, J], F32R, tag="inva", name=f"inva{eb}")
                    nc.vector.tensor_copy(inva[:], rowa_ps[:])
                    bc_ps = pb512.tile([128, J], F32, tag="b512", name=f"bc{eb}")
                    nc.tensor.matmul(bc_ps[:], sel2[:], inva[:], start=True, stop=True)
                    aa_nrm = wp.tile([128, J], F32R, tag="aanrm", bufs=3, name=f"aanrm{eb}")
                    nc.vector.tensor_mul(aa_nrm[:], aa_raw[:], bc_ps[:])

                    # seq projection for this e-block
                    sq_ps = [pproj.tile([128, NH], F32, tag="proj", name=f"sqps{eb}_{c}") for c in range(2)]
                    for hf in range(2):
                        ws = wp.tile([128, (KS // 2) * 128], F32R, tag="wseq", name=f"ws{eb}_{hf}")
                        nc.gpsimd.dma_start(
                            out=ws[:],
                            in_=seqw_ext.ap().rearrange("(kb p) e -> p kb e", p=128)[:, (KS // 2) * hf:(KS // 2) * (hf + 1), 128 * eb:128 * (eb + 1)])
                        for k in range(KS // 2):
                            kb = hf * (KS // 2) + k
                            for c in range(2):
                                nc.tensor.matmul(sq_ps[c][:], ws[:, 128 * k:128 * (k + 1)], seqT[kb][:, NH * c:NH * (c + 1)],
                                                 start=(kb == 0), stop=(kb == KS - 1))
                    seq_sb = wp.tile([128, N], F32R, tag="seq", bufs=3, name=f"seqsb{eb}")
                    for c in range(2):
                        nc.vector.tensor_scalar_add(seq_sb[:, NH * c:NH * (c + 1)], sq_ps[c][:], seqb2[:, eb:eb + 1])
                    seq_sq = wp.tile([128, N], F32R, tag="seqsq", name=f"seqsq{eb}")
                    nc.vector.tensor_mul(seq_sq[:], seq_sb[:].bitcast(F32), seq_sb[:].bitcast(F32))
                    nsq_sb = wp.tile([2, N], F32, tag="nsq", bufs=3, name=f"nsq{eb}")
                    for c in range(2):
                        nsqs_ps = psm2.tile([2, NH], F32, tag="sm2", name=f"nsqs{eb}_{c}")
                        nc.tensor.matmul(nsqs_ps[:], pairmask[:], seq_sq[:, NH * c:NH * (c + 1)], start=True, stop=True)
                        nc.vector.tensor_copy(nsq_sb[:, NH * c:NH * (c + 1)], nsqs_ps[:])
                    ibat_s = psm2.tile([128, 2 * IB], F32, tag="sm2", name=f"ibats{eb}")
                    for ib in range(IB):
                        nc.tensor.transpose(ibat_s[:, 2 * ib:2 * ib + 2], nsq_sb[:, 128 * ib:128 * (ib + 1)], eye2[:])
                    invsT = rsqrt_newton(f"s{eb}", ibat_s[:, :2 * IB], 2 * IB, 1e-4)

                    # interactions + exp-accumulate for heads 2eb, 2eb+1
                    for hh in range(2):
                        h = 2 * eb + hh
                        for ib in range(IB):
                            int_ps = pips.tile([128, J], F32, tag="ips", name=f"int{h}_{ib}")
                            nc.tensor.matmul(int_ps[:], seq_sb[64 * hh:64 * (hh + 1), 128 * ib:128 * (ib + 1)],
                                             aa_nrm[64 * hh:64 * (hh + 1), :], start=True, stop=True)
                            nc.scalar.activation(int_ps[:], int_ps[:], AF.Exp,
                                                 bias=cb[:, 0:1], scale=invsT[:, 2 * ib + hh:2 * ib + hh + 1],
                                                 accum_out=s_t[ib][:, h:h + 1])

            if stage == "eb":
                for ib in range(IB):
                    nc.sync.dma_start(out=out_ext[128 * ib:128 * (ib + 1)], in_=s_t[ib][:, 0:1])

            # ---- phase 2: finalize (batched per ACT function) ----
            if stage == "full":
                r1s = [wp.tile([128, H], F32, tag="r1", bufs=IB, name=f"r1_{ib}") for ib in range(IB)]
                pps = [wp.tile([128, 1], F32, tag="pp_t", bufs=IB, name=f"pp{ib}") for ib in range(IB)]
                for ib in range(IB):
                    nc.scalar.activation(r1s[ib][:], s_t[ib][:], AF.Ln, bias=cb[:, 1:2], scale=1.0)
                junk = wp.tile([128, H], F32, tag="junk", bufs=1, name="junk")
                for ib in range(IB):
                    nc.vector.tensor_scalar(r1s[ib][:], r1s[ib][:], 0.01, cvec[:, 0:1],
                                            op0=AL.mult, op1=AL.add)
                    nc.vector.tensor_mul(junk[:], r1s[ib][:], WV[:])
                    nc.vector.reduce_sum(pps[ib][:], junk[:], axis=mybir.AxisListType.X)
                    nc.vector.tensor_scalar_min(pps[ib][:], pps[ib][:], 80.0)
                for ib in range(IB):
                    nc.scalar.activation(pps[ib][:], pps[ib][:], AF.Exp, bias=cb[:, 2:3], scale=1.0)
                for ib in range(IB):
                    nc.scalar.activation(pps[ib][:], pps[ib][:], AF.Ln, bias=1.0, scale=1.0)
                    nc.sync.dma_start(out=out_ext[128 * ib:128 * (ib + 1)], in_=pps[ib][:])

    nc.compile()
    _GRAPH_CACHE[key] = nc
    return nc


def _prep_in_maps(inputs):
    seq_embed = np.ascontiguousarray(inputs["seq_embed"], dtype=np.float32)
    aa_embed = np.ascontiguousarray(inputs["aa_embed"], dtype=np.float32)
    ctx = np.ascontiguousarray(inputs["contextual_embed"], dtype=np.float32)
    aa_mask = np.asarray(inputs["aa_mask"])
    seq_w = np.ascontiguousarray(inputs["seq_w"], dtype=np.float32)
    seq_b = np.asarray(inputs["seq_b"], dtype=np.float32)
    aa_w = np.ascontiguousarray(inputs["aa_w"], dtype=np.float32)
    aa_b = np.asarray(inputs["aa_b"], dtype=np.float32)
    tlw = np.asarray(inputs["to_logits_w"], dtype=np.float32)
    ctx_w = np.asarray(inputs["ctx_w"], dtype=np.float32)
    ctx_b = np.asarray(inputs["ctx_b"], dtype=np.float32)
    pred_w = np.ascontiguousarray(inputs["pred_w"], dtype=np.float32)

    # permute gating space from h-major (h*32+e) to e-major (e*32+h)
    perm = (np.arange(H * H).reshape(H, H).T).reshape(-1)  # new[e*32+h] = old[h*32+e]
    ctx_wp = np.ascontiguousarray(ctx_w[:, perm])
    ctx_bp = np.ascontiguousarray(ctx_b[perm])[None, :]
    tlwT = np.ascontiguousarray(tlw.T.reshape(1, H * H))   # [1,(e h)]

    seq_b2 = np.ascontiguousarray(seq_b.reshape(EB, 128).T)
    aa_b2 = np.ascontiguousarray(aa_b.reshape(EB, 128).T)
    eye128 = np.eye(128, dtype=np.float32)
    eye2 = np.eye(2, dtype=np.float32)
    pairmask = np.zeros((128, 2), dtype=np.float32)
    pairmask[:64, 0] = 1.0
    pairmask[64:, 1] = 1.0
    sel2 = np.zeros((2, 128), dtype=np.float32)
    sel2[0, :64] = 1.0
    sel2[1, 64:] = 1.0
    ones1 = np.ones((1, 128), dtype=np.float32)

    in_maps = []
    for b in range(B):
        m = aa_mask[b].astype(np.float32)
        n_b = max(float(m.sum()), 1.0)
        cval = 0.01 * (40.0 - 2.0 * math.log(n_b))  # reference's logavgexp subtracts ln n twice
        mT = np.zeros((128, 2 * JB), dtype=np.float32)
        for c in range(JB):
            mT[:, 2 * c] = m[128 * c:128 * (c + 1)]
            mT[:, 2 * c + 1] = m[128 * c:128 * (c + 1)]
        in_maps.append({
            "seq": seq_embed[b],
            "aa": aa_embed[b],
            "seq_w": seq_w,
            "aa_w": aa_w,
            "seq_b2": seq_b2,
            "aa_b2": aa_b2,
            "ctxT": np.ascontiguousarray(ctx[b].reshape(CTX_D // 128, 128).T),
            "ctx_wp": ctx_wp,
            "ctx_bp": ctx_bp,
            "tlwT": tlwT,
            "pred_w": pred_w,
            "maskT": mT,
            "cvec": np.full((128, 1), cval, dtype=np.float32),
            "eye128": eye128,
            "eye2": eye2,
            "pairmask": pairmask,
            "sel2": sel2,
            "ones1": ones1,
        })
    return in_maps


def _run(inputs, trace=False, stage="full", n_cores=B):
    from concourse.bass_utils import run_bass_kernel_spmd
    pred_b_val = float(np.asarray(inputs["pred_b"]).reshape(-1)[0])
    nc = _build(pred_b_val, stage=stage)
    in_maps = _prep_in_maps(inputs)
    res = run_bass_kernel_spmd(nc, in_maps[:n_cores], core_ids=list(range(n_cores)), trace=trace)
    out = np.stack([res.results[c]["out"] for c in range(n_cores)], axis=0)
    return out, res


def kernel(**inputs) -> np.ndarray:
    out, _ = _run(inputs, trace=False)
    return out


# revision 12
# speedup vs baseline: 1.7487x; 1.0078x over previous
"""Trainium2 Bass kernel for the AdapterModel problem.

Data-parallel over batch: core b computes pred[b] = f(seq_embed[b], aa_embed[b], ...).
No collectives needed (B == n_cores == 8); host gathers per-core outputs.

Math per core (N=896 seq positions, J=512 aa positions, H=32 heads, D=64):
  seq_lat^T[e,i] = seq_w^T @ seq_embed^T + seq_b     (e = h*64+d, PE, f32r)
  aa_lat^T[e,j]  = aa_w^T @ aa_embed^T + aa_b
  aa_norm = aa_lat * rsqrt(sum_d aa_lat^2) * mask    (norms via pairmask matmul)
  G[h,i,j] = <seq_lat_h[:,i], aa_norm_h[:,j]>        (K=64 matmuls)
  s[i,h] = sum_j exp(G * (100*rsqrt(nsq_seq))_i - 40)    (ACT exp + accum_out)
  r[i,h] = 0.01*ln(s) + 0.01*(40 - 2*ln n)   (reference's logavgexp subtracts ln n twice)
  gating: wv = (to_logits_w .* sigmoid(ctx @ ctx_w + ctx_b)) @ pred_w
  out[i] = softplus(r[i,:] @ wv + pred_b)

ACT runs ONLY Exp during the hot loop (one table load); rsqrt is a DVE Newton
iteration (bit-trick seed), squares are DVE multiplies, sigmoid/softplus are
built from exp + reciprocal/ln, and the phase-2 Ln/Exp calls are batched by
function to avoid ACT table-set thrash (~2.7us per switch).
"""
import sys

if "/opt/trn_rl_repo" not in sys.path:
    sys.path.insert(0, "/opt/trn_rl_repo")

import math
import numpy as np

H = 32
D = 64
E = H * D            # 2048
SEQ_D = 3072
AA_D = 1280
CTX_D = 768
B, N, J = 8, 896, 512
KS = SEQ_D // 128    # 24
KA = AA_D // 128     # 10
EB = E // 128        # 16
IB = N // 128        # 7
JB = J // 128        # 4
NH = N // 2          # 448
MAGIC = 0x5F3759DF

_GRAPH_CACHE = {}


def _build(pred_b_val: float, stage: str = "full"):
    key = (float(pred_b_val), stage)
    if key in _GRAPH_CACHE:
        return _GRAPH_CACHE[key]

    import concourse.bacc as bacc
    import concourse.mybir as mybir
    import concourse.tile as tile

    F32 = mybir.dt.float32
    F32R = mybir.dt.float32r
    U32 = mybir.dt.uint32
    AF = mybir.ActivationFunctionType
    AL = mybir.AluOpType

    nc = bacc.Bacc("TRN2", target_bir_lowering=False, debug=False, num_devices=8)

    seq_ext = nc.dram_tensor("seq", [N, SEQ_D], F32, kind="ExternalInput")
    aa_ext = nc.dram_tensor("aa", [J, AA_D], F32, kind="ExternalInput")
    seqw_ext = nc.dram_tensor("seq_w", [SEQ_D, E], F32, kind="ExternalInput")
    aaw_ext = nc.dram_tensor("aa_w", [AA_D, E], F32, kind="ExternalInput")
    seqb_ext = nc.dram_tensor("seq_b2", [128, EB], F32, kind="ExternalInput")
    aab_ext = nc.dram_tensor("aa_b2", [128, EB], F32, kind="ExternalInput")
    ctxT_ext = nc.dram_tensor("ctxT", [128, CTX_D // 128], F32, kind="ExternalInput")
    ctxw_ext = nc.dram_tensor("ctx_wp", [CTX_D, H * H], F32, kind="ExternalInput")
    ctxb_ext = nc.dram_tensor("ctx_bp", [1, H * H], F32, kind="ExternalInput")
    tlw_ext = nc.dram_tensor("tlwT", [1, H * H], F32, kind="ExternalInput")
    predw_ext = nc.dram_tensor("pred_w", [H, 1], F32, kind="ExternalInput")
    maskT_ext = nc.dram_tensor("maskT", [128, 2 * JB], F32, kind="ExternalInput")
    cvec_ext = nc.dram_tensor("cvec", [128, 1], F32, kind="ExternalInput")
    eye_ext = nc.dram_tensor("eye128", [128, 128], F32, kind="ExternalInput")
    eye2_ext = nc.dram_tensor("eye2", [2, 2], F32, kind="ExternalInput")
    pm_ext = nc.dram_tensor("pairmask", [128, 2], F32, kind="ExternalInput")
    sel2_ext = nc.dram_tensor("sel2", [2, 128], F32, kind="ExternalInput")
    ones1_ext = nc.dram_tensor("ones1", [1, 128], F32, kind="ExternalInput")
    out_ext = nc.dram_tensor("out", [N], F32, kind="ExternalOutput")

    KCTX = CTX_D // 128  # 6

    with tile.TileContext(nc) as tc:
        with tc.tile_pool(name="persist", bufs=1) as pp, \
             tc.tile_pool(name="work", bufs=2) as wp, \
             tc.tile_pool(name="pproj", bufs=2, space="PSUM") as pproj, \
             tc.tile_pool(name="pb512", bufs=2, space="PSUM") as pb512, \
             tc.tile_pool(name="pips", bufs=3, space="PSUM") as pips, \
             tc.tile_pool(name="psm2", bufs=1, space="PSUM") as psm2:

            def dummy_out(src_ap):
                dummy = pp.tile([128, 1], F32, tag="dummy", name="dummy")
                nc.vector.tensor_copy(dummy[:], src_ap)
                for ib in range(IB):
                    nc.sync.dma_start(out=out_ext[128 * ib:128 * (ib + 1)], in_=dummy[:])

            # ---- constants / tiny inputs ----
            eye = pp.tile([128, 128], F32, tag="eye")
            nc.sync.dma_start(out=eye[:], in_=eye_ext[:])
            eyer = pp.tile([128, 128], F32R, tag="eyer")
            nc.gpsimd.dma_start(out=eyer[:], in_=eye_ext[:])
            eye2 = pp.tile([2, 2], F32, tag="eye2")
            nc.sync.dma_start(out=eye2[:], in_=eye2_ext[:])
            pairmask = pp.tile([128, 2], F32R, tag="pairmask")
            nc.gpsimd.dma_start(out=pairmask[:], in_=pm_ext[:])
            sel2 = pp.tile([2, 128], F32R, tag="sel2")
            nc.gpsimd.dma_start(out=sel2[:], in_=sel2_ext[:])
            ones1 = pp.tile([1, 128], F32R, tag="ones1")
            nc.gpsimd.dma_start(out=ones1[:], in_=ones1_ext[:])
            maskT = pp.tile([128, 2 * JB], F32, tag="maskT")
            nc.sync.dma_start(out=maskT[:], in_=maskT_ext[:])
            cvec = pp.tile([128, 1], F32, tag="cvec")
            nc.sync.dma_start(out=cvec[:], in_=cvec_ext[:])
            seqb2 = pp.tile([128, EB], F32, tag="seqb2")
            nc.sync.dma_start(out=seqb2[:], in_=seqb_ext[:])
            aab2 = pp.tile([128, EB], F32, tag="aab2")
            nc.sync.dma_start(out=aab2[:], in_=aab_ext[:])
            # const bias columns: [0]=-40, [1]=1e-30, [2]=pred_b
            cb = pp.tile([128, 3], F32, tag="cb")
            nc.gpsimd.memset(cb[:, 0:1], -40.0)
            nc.gpsimd.memset(cb[:, 1:2], 1e-30)
            nc.gpsimd.memset(cb[:, 2:3], float(pred_b_val))
            magic = pp.tile([128, 16], U32, tag="magic")
            nc.gpsimd.memset(magic[:], MAGIC)

            def rsqrt_newton(name, xin_ap, w, scale_mul):
                """SBUF f32 [128, w] <- rsqrt(xin * scale_mul); xin may be PSUM."""
                x = wp.tile([128, 16], F32, tag="nwx", name=f"nwx{name}")
                nc.vector.tensor_scalar(x[:, :w], xin_ap, scale_mul, 1e-35, op0=AL.mult, op1=AL.add)
                u = wp.tile([128, 16], U32, tag="nwu", name=f"nwu{name}")
                nc.vector.tensor_scalar(u[:, :w], x[:, :w].bitcast(U32), 1, None, op0=AL.logical_shift_right)
                y0u = wp.tile([128, 16], U32, tag="nwy0", name=f"nwy0{name}")
                nc.vector.tensor_tensor(out=y0u[:, :w], in0=magic[:, :w], in1=u[:, :w], op=AL.subtract)
                y = wp.tile([128, 16], F32, tag="nwy", name=f"nwy{name}")
                t = wp.tile([128, 16], F32, tag="nwt", name=f"nwt{name}")
                nc.vector.tensor_mul(t[:, :w], y0u[:, :w].bitcast(F32), y0u[:, :w].bitcast(F32))
                nc.vector.tensor_mul(t[:, :w], t[:, :w], x[:, :w])
                nc.vector.tensor_scalar(t[:, :w], t[:, :w], -0.5, 1.5, op0=AL.mult, op1=AL.add)
                nc.vector.tensor_mul(y[:, :w], y0u[:, :w].bitcast(F32), t[:, :w])
                nc.vector.tensor_mul(t[:, :w], y[:, :w], y[:, :w])
                nc.vector.tensor_mul(t[:, :w], t[:, :w], x[:, :w])
                nc.vector.tensor_scalar(t[:, :w], t[:, :w], -0.5, 1.5, op0=AL.mult, op1=AL.add)
                nc.vector.tensor_mul(y[:, :w], y[:, :w], t[:, :w])
                return y

            # ---- gating chain ----
            ctxT = pp.tile([128, KCTX], F32R, tag="ctxT")
            nc.gpsimd.dma_start(out=ctxT[:], in_=ctxT_ext[:])
            g_ps = [pips.tile([1, 512], F32, tag="ips", name=f"gps{i}") for i in range(2)]
            for c in range(KCTX):
                wctx = wp.tile([128, H * H], F32R, tag="wseq", name=f"wctx{c}")
                nc.gpsimd.dma_start(out=wctx[:], in_=ctxw_ext[128 * c:128 * (c + 1), :])
                for half in range(2):
                    nc.tensor.matmul(g_ps[half][:], ctxT[:, c:c + 1], wctx[:, 512 * half:512 * (half + 1)],
                                     start=(c == 0), stop=(c == KCTX - 1))
            ctxb = pp.tile([1, H * H], F32, tag="ctxb")
            nc.sync.dma_start(out=ctxb[:], in_=ctxb_ext[:])
            g_sb = pp.tile([1, H * H], F32, tag="g_sb")
            for half in range(2):
                nc.vector.tensor_add(g_sb[:, 512 * half:512 * (half + 1)], g_ps[half][:], ctxb[:, 512 * half:512 * (half + 1)])
            # sigmoid(x) = 1/(1+exp(-x))
            sig = pp.tile([1, H * H], F32, tag="sig")
            nc.scalar.activation(sig[:], g_sb[:], AF.Exp, bias=0.0, scale=-1.0)
            nc.vector.tensor_scalar_add(sig[:], sig[:], 1.0)
            nc.vector.reciprocal(sig[:], sig[:])
            tlw = pp.tile([1, H * H], F32, tag="tlw")
            nc.sync.dma_start(out=tlw[:], in_=tlw_ext[:])
            nc.vector.tensor_mul(sig[:], sig[:], tlw[:])   # w_b^T flat, e-major
            wb_dram = nc.dram_tensor("wb_bounce", [H, H], F32)
            nc.sync.dma_start(out=wb_dram.ap().rearrange("e h -> (e h)")[None, :], in_=sig[:])
            wbT = pp.tile([H, H], F32R, tag="wbT")
            nc.gpsimd.dma_start(out=wbT[:], in_=wb_dram[:])
            predw = pp.tile([H, 1], F32R, tag="predw")
            nc.gpsimd.dma_start(out=predw[:], in_=predw_ext[:])
            wv_ps = pips.tile([1, H], F32, tag="ips")
            nc.tensor.matmul(wv_ps[:], predw[:], wbT[:], start=True, stop=True)
            wv_sb = pp.tile([1, H], F32R, tag="wv_sb")
            nc.vector.tensor_copy(wv_sb[:], wv_ps[:])
            WV_ps = pips.tile([128, H], F32, tag="ips")
            nc.tensor.matmul(WV_ps[:], ones1[:], wv_sb[:], start=True, stop=True)
            WV = pp.tile([128, H], F32, tag="WV")
            nc.vector.tensor_copy(WV[:], WV_ps[:])

            if stage == "gating":
                dummy_out(WV[:, 0:1])

            # ---- phase 0: transpose aa_embed and seq_embed ----
            if stage in ("tpose", "eb", "full"):
                aaT = [pp.tile([128, J], F32R, tag=f"aat{kb}", name=f"aat{kb}") for kb in range(KA)]
                for jb in range(JB):
                    nat = wp.tile([128, AA_D], F32R, tag="nat", bufs=2, name=f"anat{jb}")
                    nc.gpsimd.dma_start(out=nat[:], in_=aa_ext[128 * jb:128 * (jb + 1), :])
                    for kb in range(KA):
                        tp = pips.tile([128, 128], F32R, tag="ips", name=f"atp{jb}_{kb}")
                        nc.tensor.transpose(tp[:], nat[:, 128 * kb:128 * (kb + 1)], eyer[:])
                        nc.vector.tensor_copy(aaT[kb][:, 128 * jb:128 * (jb + 1)], tp[:])

                seqT = [pp.tile([128, N], F32R, tag=f"ast{kb}", name=f"ast{kb}") for kb in range(KS)]
                for ib in range(IB):
                    for hf in range(2):
                        nat = wp.tile([128, SEQ_D // 2], F32R, tag="nat", bufs=2, name=f"snat{ib}_{hf}")
                        nc.gpsimd.dma_start(out=nat[:], in_=seq_ext[128 * ib:128 * (ib + 1), (SEQ_D // 2) * hf:(SEQ_D // 2) * (hf + 1)])
                        for k in range(KS // 2):
                            kb = hf * (KS // 2) + k
                            tp = pips.tile([128, 128], F32R, tag="ips", name=f"stp{ib}_{kb}")
                            nc.tensor.transpose(tp[:], nat[:, 128 * k:128 * (k + 1)], eyer[:])
                            nc.vector.tensor_copy(seqT[kb][:, 128 * ib:128 * (ib + 1)], tp[:])

            if stage == "tpose":
                dummy_out(seqT[0][:, 0:1].bitcast(F32))

            # ---- phase 1: per head-pair block ----
            if stage in ("eb", "full"):
                s_t = [pp.tile([128, H], F32, tag=f"s_t{ib}", name=f"s_t{ib}") for ib in range(IB)]

                for eb in range(EB):
                    # aa projection for this e-block
                    aa_ps = pproj.tile([128, J], F32, tag="proj", name=f"aaps{eb}")
                    waa = wp.tile([128, KA * 128], F32R, tag="waa", name=f"waa{eb}")
                    nc.gpsimd.dma_start(
                        out=waa[:],
                        in_=aaw_ext.ap().rearrange("(kb p) e -> p kb e", p=128)[:, :, 128 * eb:128 * (eb + 1)])
                    for kb in range(KA):
                        nc.tensor.matmul(aa_ps[:], waa[:, 128 * kb:128 * (kb + 1)], aaT[kb][:],
                                         start=(kb == 0), stop=(kb == KA - 1))
                    aa_raw = wp.tile([128, J], F32, tag="araw", name=f"araw{eb}")
                    nc.vector.tensor_scalar_add(aa_raw[:], aa_ps[:], aab2[:, eb:eb + 1])
                    aa_sq = wp.tile([128, N], F32R, tag="seqsq", name=f"aasq{eb}")
                    nc.vector.tensor_mul(aa_sq[:, :J], aa_raw[:], aa_raw[:])
                    nsqa_ps = pb512.tile([2, J], F32, tag="b512", name=f"nsqa{eb}")
                    nc.tensor.matmul(nsqa_ps[:], pairmask[:], aa_sq[:, :J], start=True, stop=True)
                    nsqa_sb = wp.tile([2, J], F32, tag="nsq", bufs=3, name=f"nsqa_sb{eb}")
                    nc.vector.tensor_copy(nsqa_sb[:], nsqa_ps[:])
                    ibat_a = psm2.tile([128, 2 * JB], F32, tag="sm2", name=f"ibata{eb}")
                    for c in range(JB):
                        nc.tensor.transpose(ibat_a[:, 2 * c:2 * c + 2], nsqa_sb[:, 128 * c:128 * (c + 1)], eye2[:])
                    ya = rsqrt_newton(f"a{eb}", ibat_a[:, :2 * JB], 2 * JB, 1.0)
                    nc.vector.tensor_mul(ya[:, :2 * JB], ya[:, :2 * JB], maskT[:])
                    rowa_ps = pb512.tile([2, J], F32, tag="b512", name=f"rowa{eb}")
                    for c in range(JB):
                        nc.tensor.transpose(rowa_ps[:, 128 * c:128 * (c + 1)], ya[:, 2 * c:2 * c + 2], eye[:])
                    inva = wp.tile([2, J], F32R, tag="inva", name=f"inva{eb}")
                    nc.vector.tensor_copy(inva[:], rowa_ps[:])
                    bc_ps = pb512.tile([128, J], F32, tag="b512", name=f"bc{eb}")
                    nc.tensor.matmul(bc_ps[:], sel2[:], inva[:], start=True, stop=True)
                    aa_nrm = wp.tile([128, J], F32R, tag="aanrm", bufs=3, name=f"aanrm{eb}")
                    nc.vector.tensor_mul(aa_nrm[:], aa_raw[:], bc_ps[:])

                    # seq projection for this e-block
                    sq_ps = [pproj.tile([128, NH], F32, tag="proj", name=f"sqps{eb}_{c}") for c in range(2)]
                    for hf in range(2):
                        ws = wp.tile([128, (KS // 2) * 128], F32R, tag="wseq", name=f"ws{eb}_{hf}")
                        nc.gpsimd.dma_start(
                            out=ws[:],
                            in_=seqw_ext.ap().rearrange("(kb p) e -> p kb e", p=128)[:, (KS // 2) * hf:(KS // 2) * (hf + 1), 128 * eb:128 * (eb + 1)])
                        for k in range(KS // 2):
                            kb = hf * (KS // 2) + k
                            for c in range(2):
                                nc.tensor.matmul(sq_ps[c][:], ws[:, 128 * k:128 * (k + 1)], seqT[kb][:, NH * c:NH * (c + 1)],
                                                 start=(kb == 0), stop=(kb == KS - 1))
                    seq_sb = wp.tile([128, N], F32R, tag="seq", bufs=3, name=f"seqsb{eb}")
                    for c in range(2):
                        nc.vector.tensor_scalar_add(seq_sb[:, NH * c:NH * (c + 1)], sq_ps[c][:], seqb2[:, eb:eb + 1])
                    seq_sq = wp.tile([128, N], F32R, tag="seqsq", name=f"seqsq{eb}")
                    nc.vector.tensor_mul(seq_sq[:], seq_sb[:].bitcast(F32), seq_sb[:].bitcast(F32))
                    nsq_sb = wp.tile([2, N], F32, tag="nsq", bufs=3, name=f"nsq{eb}")
                    for c in range(2):
                        nsqs_ps = psm2.tile([2, NH], F32, tag="sm2", name=f"nsqs{eb}_{c}")
                        nc.tensor.matmul(nsqs_ps[:], pairmask[:], seq_sq[:, NH * c:NH * (c + 1)], start=True, stop=True)
                        nc.vector.tensor_copy(nsq_sb[:, NH * c:NH * (c + 1)], nsqs_ps[:])
                    ibat_s = psm2.tile([128, 2 * IB], F32, tag="sm2", name=f"ibats{eb}")
                    for ib in range(IB):
                        nc.tensor.transpose(ibat_s[:, 2 * ib:2 * ib + 2], nsq_sb[:, 128 * ib:128 * (ib + 1)], eye2[:])
                    invsT = rsqrt_newton(f"s{eb}", ibat_s[:, :2 * IB], 2 * IB, 1e-4)

                    # interactions + exp-accumulate for heads 2eb, 2eb+1
                    # hh inner: alternating head halves use disjoint PE row groups,
                    # letting the next LDWEIGHTS overlap the running matmul
                    for ib in range(IB):
                        for hh in range(2):
                            h = 2 * eb + hh
                            int_ps = pips.tile([128, J], F32, tag="ips", name=f"int{h}_{ib}")
                            nc.tensor.matmul(int_ps[:], seq_sb[64 * hh:64 * (hh + 1), 128 * ib:128 * (ib + 1)],
                                             aa_nrm[64 * hh:64 * (hh + 1), :], start=True, stop=True)
                            nc.scalar.activation(int_ps[:], int_ps[:], AF.Exp,
                                                 bias=cb[:, 0:1], scale=invsT[:, 2 * ib + hh:2 * ib + hh + 1],
                                                 accum_out=s_t[ib][:, h:h + 1])

            if stage == "eb":
                for ib in range(IB):
                    nc.sync.dma_start(out=out_ext[128 * ib:128 * (ib + 1)], in_=s_t[ib][:, 0:1])

            # ---- phase 2: finalize (batched per ACT function) ----
            if stage == "full":
                r1s = [wp.tile([128, H], F32, tag="r1", bufs=IB, name=f"r1_{ib}") for ib in range(IB)]
                pps = [wp.tile([128, 1], F32, tag="pp_t", bufs=IB, name=f"pp{ib}") for ib in range(IB)]
                for ib in range(IB):
                    nc.scalar.activation(r1s[ib][:], s_t[ib][:], AF.Ln, bias=cb[:, 1:2], scale=1.0)
                junk = wp.tile([128, H], F32, tag="junk", bufs=1, name="junk")
                for ib in range(IB):
                    nc.vector.tensor_scalar(r1s[ib][:], r1s[ib][:], 0.01, cvec[:, 0:1],
                                            op0=AL.mult, op1=AL.add)
                    nc.vector.tensor_mul(junk[:], r1s[ib][:], WV[:])
                    nc.vector.reduce_sum(pps[ib][:], junk[:], axis=mybir.AxisListType.X)
                    nc.vector.tensor_scalar_min(pps[ib][:], pps[ib][:], 80.0)
                for ib in range(IB):
                    nc.scalar.activation(pps[ib][:], pps[ib][:], AF.Exp, bias=cb[:, 2:3], scale=1.0)
                for ib in range(IB):
                    nc.scalar.activation(pps[ib][:], pps[ib][:], AF.Ln, bias=1.0, scale=1.0)
                    nc.sync.dma_start(out=out_ext[128 * ib:128 * (ib + 1)], in_=pps[ib][:])

    nc.compile()
    _GRAPH_CACHE[key] = nc
    return nc


def _prep_in_maps(inputs):
    seq_embed = np.ascontiguousarray(inputs["seq_embed"], dtype=np.float32)
    aa_embed = np.ascontiguousarray(inputs["aa_embed"], dtype=np.float32)
    ctx = np.ascontiguousarray(inputs["contextual_embed"], dtype=np.float32)
    aa_mask = np.asarray(inputs["aa_mask"])
    seq_w = np.ascontiguousarray(inputs["seq_w"], dtype=np.float32)
    seq_b = np.asarray(inputs["seq_b"], dtype=np.float32)
    aa_w = np.ascontiguousarray(inputs["aa_w"], dtype=np.float32)
    aa_b = np.asarray(inputs["aa_b"], dtype=np.float32)
    tlw = np.asarray(inputs["to_logits_w"], dtype=np.float32)
    ctx_w = np.asarray(inputs["ctx_w"], dtype=np.float32)
    ctx_b = np.asarray(inputs["ctx_b"], dtype=np.float32)
    pred_w = np.ascontiguousarray(inputs["pred_w"], dtype=np.float32)

    # permute gating space from h-major (h*32+e) to e-major (e*32+h)
    perm = (np.arange(H * H).reshape(H, H).T).reshape(-1)  # new[e*32+h] = old[h*32+e]
    ctx_wp = np.ascontiguousarray(ctx_w[:, perm])
    ctx_bp = np.ascontiguousarray(ctx_b[perm])[None, :]
    tlwT = np.ascontiguousarray(tlw.T.reshape(1, H * H))   # [1,(e h)]

    seq_b2 = np.ascontiguousarray(seq_b.reshape(EB, 128).T)
    aa_b2 = np.ascontiguousarray(aa_b.reshape(EB, 128).T)
    eye128 = np.eye(128, dtype=np.float32)
    eye2 = np.eye(2, dtype=np.float32)
    pairmask = np.zeros((128, 2), dtype=np.float32)
    pairmask[:64, 0] = 1.0
    pairmask[64:, 1] = 1.0
    sel2 = np.zeros((2, 128), dtype=np.float32)
    sel2[0, :64] = 1.0
    sel2[1, 64:] = 1.0
    ones1 = np.ones((1, 128), dtype=np.float32)

    in_maps = []
    for b in range(B):
        m = aa_mask[b].astype(np.float32)
        n_b = max(float(m.sum()), 1.0)
        cval = 0.01 * (40.0 - 2.0 * math.log(n_b))  # reference's logavgexp subtracts ln n twice
        mT = np.zeros((128, 2 * JB), dtype=np.float32)
        for c in range(JB):
            mT[:, 2 * c] = m[128 * c:128 * (c + 1)]
            mT[:, 2 * c + 1] = m[128 * c:128 * (c + 1)]
        in_maps.append({
            "seq": seq_embed[b],
            "aa": aa_embed[b],
            "seq_w": seq_w,
            "aa_w": aa_w,
            "seq_b2": seq_b2,
            "aa_b2": aa_b2,
            "ctxT": np.ascontiguousarray(ctx[b].reshape(CTX_D // 128, 128).T),
            "ctx_wp": ctx_wp,
            "ctx_bp": ctx_bp,
            "tlwT": tlwT,
            "pred_w": pred_w,
            "maskT": mT,
            "cvec": np.full((128, 1), cval, dtype=np.float32),
            "eye128": eye128,
            "eye2": eye2,
            "pairmask": pairmask,
            "sel2": sel2,
            "ones1": ones1,
        })
    return in_maps


def _run(inputs, trace=False, stage="full", n_cores=B):
    from concourse.bass_utils import run_bass_kernel_spmd
    pred_b_val = float(np.asarray(inputs["pred_b"]).reshape(-1)[0])
    nc = _build(pred_b_val, stage=stage)
    in_maps = _prep_in_maps(inputs)
    res = run_bass_kernel_spmd(nc, in_maps[:n_cores], core_ids=list(range(n_cores)), trace=trace)
    out = np.stack([res.results[c]["out"] for c in range(n_cores)], axis=0)
    return out, res


def kernel(**inputs) -> np.ndarray:
    out, _ = _run(inputs, trace=False)
    return out


# revision 13
# speedup vs baseline: 1.7895x; 1.0233x over previous
"""Trainium2 Bass kernel for the AdapterModel problem.

Data-parallel over batch: core b computes pred[b] = f(seq_embed[b], aa_embed[b], ...).
No collectives needed (B == n_cores == 8); host gathers per-core outputs.

Math per core (N=896 seq positions, J=512 aa positions, H=32 heads, D=64):
  seq_lat^T[e,i] = seq_w^T @ seq_embed^T + seq_b     (e = h*64+d, PE, f32r)
  aa_lat^T[e,j]  = aa_w^T @ aa_embed^T + aa_b
  aa_norm = aa_lat * rsqrt(sum_d aa_lat^2) * mask    (norms via pairmask matmul)
  G[h,i,j] = <seq_lat_h[:,i], aa_norm_h[:,j]>        (K=64 matmuls)
  s[i,h] = sum_j exp(G * (100*rsqrt(nsq_seq))_i - 40)    (ACT exp + accum_out)
  r[i,h] = 0.01*ln(s) + 0.01*(40 - 2*ln n)   (reference's logavgexp subtracts ln n twice)
  gating: wv = (to_logits_w .* sigmoid(ctx @ ctx_w + ctx_b)) @ pred_w
  out[i] = softplus(r[i,:] @ wv + pred_b)

ACT runs ONLY Exp during the hot loop (one table load); rsqrt is a DVE Newton
iteration (bit-trick seed), squares are DVE multiplies, sigmoid/softplus are
built from exp + reciprocal/ln, and the phase-2 Ln/Exp calls are batched by
function to avoid ACT table-set thrash (~2.7us per switch).
"""
import sys

if "/opt/trn_rl_repo" not in sys.path:
    sys.path.insert(0, "/opt/trn_rl_repo")

import math
import numpy as np

H = 32
D = 64
E = H * D            # 2048
SEQ_D = 3072
AA_D = 1280
CTX_D = 768
B, N, J = 8, 896, 512
KS = SEQ_D // 128    # 24
KA = AA_D // 128     # 10
EB = E // 128        # 16
IB = N // 128        # 7
JB = J // 128        # 4
NH = N // 2          # 448
MAGIC = 0x5F3759DF

_GRAPH_CACHE = {}


def _build(pred_b_val: float, stage: str = "full"):
    key = (float(pred_b_val), stage)
    if key in _GRAPH_CACHE:
        return _GRAPH_CACHE[key]

    import concourse.bacc as bacc
    import concourse.mybir as mybir
    import concourse.tile as tile

    F32 = mybir.dt.float32
    F32R = mybir.dt.float32r
    U32 = mybir.dt.uint32
    AF = mybir.ActivationFunctionType
    AL = mybir.AluOpType

    nc = bacc.Bacc("TRN2", target_bir_lowering=False, debug=False, num_devices=8)

    seq_ext = nc.dram_tensor("seq", [N, SEQ_D], F32, kind="ExternalInput")
    aa_ext = nc.dram_tensor("aa", [J, AA_D], F32, kind="ExternalInput")
    seqw_ext = nc.dram_tensor("seq_w", [SEQ_D, E], F32, kind="ExternalInput")
    aaw_ext = nc.dram_tensor("aa_w", [AA_D, E], F32, kind="ExternalInput")
    seqb_ext = nc.dram_tensor("seq_b2", [128, EB], F32, kind="ExternalInput")
    aab_ext = nc.dram_tensor("aa_b2", [128, EB], F32, kind="ExternalInput")
    ctxT_ext = nc.dram_tensor("ctxT", [128, CTX_D // 128], F32, kind="ExternalInput")
    ctxw_ext = nc.dram_tensor("ctx_wp", [CTX_D, H * H], F32, kind="ExternalInput")
    ctxb_ext = nc.dram_tensor("ctx_bp", [1, H * H], F32, kind="ExternalInput")
    tlw_ext = nc.dram_tensor("tlwT", [1, H * H], F32, kind="ExternalInput")
    predw_ext = nc.dram_tensor("pred_w", [H, 1], F32, kind="ExternalInput")
    maskT_ext = nc.dram_tensor("maskT", [128, 2 * JB], F32, kind="ExternalInput")
    cvec_ext = nc.dram_tensor("cvec", [128, 1], F32, kind="ExternalInput")
    eye_ext = nc.dram_tensor("eye128", [128, 128], F32, kind="ExternalInput")
    eye2_ext = nc.dram_tensor("eye2", [2, 2], F32, kind="ExternalInput")
    pm_ext = nc.dram_tensor("pairmask", [128, 2], F32, kind="ExternalInput")
    sel2_ext = nc.dram_tensor("sel2", [2, 128], F32, kind="ExternalInput")
    ones1_ext = nc.dram_tensor("ones1", [1, 128], F32, kind="ExternalInput")
    out_ext = nc.dram_tensor("out", [N], F32, kind="ExternalOutput")

    KCTX = CTX_D // 128  # 6

    with tile.TileContext(nc) as tc:
        with tc.tile_pool(name="persist", bufs=1) as pp, \
             tc.tile_pool(name="work", bufs=2) as wp, \
             tc.tile_pool(name="pproj", bufs=2, space="PSUM") as pproj, \
             tc.tile_pool(name="pb512", bufs=2, space="PSUM") as pb512, \
             tc.tile_pool(name="pips", bufs=3, space="PSUM") as pips, \
             tc.tile_pool(name="psm2", bufs=1, space="PSUM") as psm2:

            def dummy_out(src_ap):
                dummy = pp.tile([128, 1], F32, tag="dummy", name="dummy")
                nc.vector.tensor_copy(dummy[:], src_ap)
                for ib in range(IB):
                    nc.sync.dma_start(out=out_ext[128 * ib:128 * (ib + 1)], in_=dummy[:])

            # ---- constants / tiny inputs ----
            eye = pp.tile([128, 128], F32, tag="eye")
            nc.sync.dma_start(out=eye[:], in_=eye_ext[:])
            eyer = pp.tile([128, 128], F32R, tag="eyer")
            nc.gpsimd.dma_start(out=eyer[:], in_=eye_ext[:])
            eye2 = pp.tile([2, 2], F32, tag="eye2")
            nc.sync.dma_start(out=eye2[:], in_=eye2_ext[:])
            pairmask = pp.tile([128, 2], F32R, tag="pairmask")
            nc.gpsimd.dma_start(out=pairmask[:], in_=pm_ext[:])
            sel2 = pp.tile([2, 128], F32R, tag="sel2")
            nc.gpsimd.dma_start(out=sel2[:], in_=sel2_ext[:])
            ones1 = pp.tile([1, 128], F32R, tag="ones1")
            nc.gpsimd.dma_start(out=ones1[:], in_=ones1_ext[:])
            maskT = pp.tile([128, 2 * JB], F32, tag="maskT")
            nc.sync.dma_start(out=maskT[:], in_=maskT_ext[:])
            cvec = pp.tile([128, 1], F32, tag="cvec")
            nc.sync.dma_start(out=cvec[:], in_=cvec_ext[:])
            seqb2 = pp.tile([128, EB], F32, tag="seqb2")
            nc.sync.dma_start(out=seqb2[:], in_=seqb_ext[:])
            aab2 = pp.tile([128, EB], F32, tag="aab2")
            nc.sync.dma_start(out=aab2[:], in_=aab_ext[:])
            # const bias columns: [0]=-40, [1]=1e-30, [2]=pred_b
            cb = pp.tile([128, 3], F32, tag="cb")
            nc.gpsimd.memset(cb[:, 0:1], -40.0)
            nc.gpsimd.memset(cb[:, 1:2], 1e-30)
            nc.gpsimd.memset(cb[:, 2:3], float(pred_b_val))
            magic = pp.tile([128, 16], U32, tag="magic")
            nc.gpsimd.memset(magic[:], MAGIC)

            def rsqrt_newton(name, xin_ap, w, scale_mul):
                """SBUF f32 [128, w] <- rsqrt(xin * scale_mul); xin may be PSUM."""
                x = wp.tile([128, 16], F32, tag="nwx", name=f"nwx{name}")
                nc.vector.tensor_scalar(x[:, :w], xin_ap, scale_mul, 1e-35, op0=AL.mult, op1=AL.add)
                u = wp.tile([128, 16], U32, tag="nwu", name=f"nwu{name}")
                nc.vector.tensor_scalar(u[:, :w], x[:, :w].bitcast(U32), 1, None, op0=AL.logical_shift_right)
                y0u = wp.tile([128, 16], U32, tag="nwy0", name=f"nwy0{name}")
                nc.vector.tensor_tensor(out=y0u[:, :w], in0=magic[:, :w], in1=u[:, :w], op=AL.subtract)
                y = wp.tile([128, 16], F32, tag="nwy", name=f"nwy{name}")
                t = wp.tile([128, 16], F32, tag="nwt", name=f"nwt{name}")
                nc.vector.tensor_mul(t[:, :w], y0u[:, :w].bitcast(F32), y0u[:, :w].bitcast(F32))
                nc.vector.tensor_mul(t[:, :w], t[:, :w], x[:, :w])
                nc.vector.tensor_scalar(t[:, :w], t[:, :w], -0.5, 1.5, op0=AL.mult, op1=AL.add)
                nc.vector.tensor_mul(y[:, :w], y0u[:, :w].bitcast(F32), t[:, :w])
                nc.vector.tensor_mul(t[:, :w], y[:, :w], y[:, :w])
                nc.vector.tensor_mul(t[:, :w], t[:, :w], x[:, :w])
                nc.vector.tensor_scalar(t[:, :w], t[:, :w], -0.5, 1.5, op0=AL.mult, op1=AL.add)
                nc.vector.tensor_mul(y[:, :w], y[:, :w], t[:, :w])
                return y

            # ---- gating chain ----
            ctxT = pp.tile([128, KCTX], F32R, tag="ctxT")
            nc.gpsimd.dma_start(out=ctxT[:], in_=ctxT_ext[:])
            g_ps = [pips.tile([1, 512], F32, tag="ips", name=f"gps{i}") for i in range(2)]
            for c in range(KCTX):
                wctx = wp.tile([128, H * H], F32R, tag="wseq", name=f"wctx{c}")
                nc.gpsimd.dma_start(out=wctx[:], in_=ctxw_ext[128 * c:128 * (c + 1), :])
                for half in range(2):
                    nc.tensor.matmul(g_ps[half][:], ctxT[:, c:c + 1], wctx[:, 512 * half:512 * (half + 1)],
                                     start=(c == 0), stop=(c == KCTX - 1))
            ctxb = pp.tile([1, H * H], F32, tag="ctxb")
            nc.sync.dma_start(out=ctxb[:], in_=ctxb_ext[:])
            g_sb = pp.tile([1, H * H], F32, tag="g_sb")
            for half in range(2):
                nc.vector.tensor_add(g_sb[:, 512 * half:512 * (half + 1)], g_ps[half][:], ctxb[:, 512 * half:512 * (half + 1)])
            # sigmoid(x) = 1/(1+exp(-x))
            sig = pp.tile([1, H * H], F32, tag="sig")
            nc.scalar.activation(sig[:], g_sb[:], AF.Exp, bias=0.0, scale=-1.0)
            nc.vector.tensor_scalar_add(sig[:], sig[:], 1.0)
            nc.vector.reciprocal(sig[:], sig[:])
            tlw = pp.tile([1, H * H], F32, tag="tlw")
            nc.sync.dma_start(out=tlw[:], in_=tlw_ext[:])
            nc.vector.tensor_mul(sig[:], sig[:], tlw[:])   # w_b^T flat, e-major
            wb_dram = nc.dram_tensor("wb_bounce", [H, H], F32)
            nc.sync.dma_start(out=wb_dram.ap().rearrange("e h -> (e h)")[None, :], in_=sig[:])
            wbT = pp.tile([H, H], F32R, tag="wbT")
            nc.gpsimd.dma_start(out=wbT[:], in_=wb_dram[:])
            predw = pp.tile([H, 1], F32R, tag="predw")
            nc.gpsimd.dma_start(out=predw[:], in_=predw_ext[:])
            WV = pp.tile([128, H], F32, tag="WV")

            def gating_tail():
                # PE is in-order: emit these after the hot loop so the DRAM
                # bounce + sigmoid chain never stalls phase 0/1 matmuls
                wv_ps = pips.tile([1, H], F32, tag="ips")
                nc.tensor.matmul(wv_ps[:], predw[:], wbT[:], start=True, stop=True)
                wv_sb = pp.tile([1, H], F32R, tag="wv_sb")
                nc.vector.tensor_copy(wv_sb[:], wv_ps[:])
                WV_ps = pips.tile([128, H], F32, tag="ips")
                nc.tensor.matmul(WV_ps[:], ones1[:], wv_sb[:], start=True, stop=True)
                nc.vector.tensor_copy(WV[:], WV_ps[:])

            if stage == "gating":
                gating_tail()
                dummy_out(WV[:, 0:1])

            # ---- phase 0: transpose aa_embed and seq_embed ----
            if stage in ("tpose", "eb", "full"):
                aaT = [pp.tile([128, J], F32R, tag=f"aat{kb}", name=f"aat{kb}") for kb in range(KA)]
                tpsel = 0
                for jb in range(JB):
                    for hf in range(2):
                        w0 = 640 * hf
                        kw = 5
                        nat = wp.tile([128, 768], F32R, tag="nat", bufs=4, name=f"anat{jb}_{hf}")
                        nc.gpsimd.dma_start(out=nat[:, :640], in_=aa_ext[128 * jb:128 * (jb + 1), w0:w0 + 640])
                        for k in range(kw):
                            kb = hf * kw + k
                            pool = pips if (tpsel % 2 == 0) else pproj
                            tpsel += 1
                            tp = pool.tile([128, 128], F32R, tag="ips" if pool is pips else "proj", name=f"atp{jb}_{kb}")
                            nc.tensor.transpose(tp[:], nat[:, 128 * k:128 * (k + 1)], eyer[:])
                            nc.vector.tensor_copy(aaT[kb][:, 128 * jb:128 * (jb + 1)], tp[:])

                seqT = [pp.tile([128, N], F32R, tag=f"ast{kb}", name=f"ast{kb}") for kb in range(KS)]
                for ib in range(IB):
                    for hf in range(4):
                        nat = wp.tile([128, 768], F32R, tag="nat", bufs=4, name=f"snat{ib}_{hf}")
                        nc.gpsimd.dma_start(out=nat[:], in_=seq_ext[128 * ib:128 * (ib + 1), 768 * hf:768 * (hf + 1)])
                        for k in range(6):
                            kb = hf * 6 + k
                            pool = pips if (tpsel % 2 == 0) else pproj
                            tpsel += 1
                            tp = pool.tile([128, 128], F32R, tag="ips" if pool is pips else "proj", name=f"stp{ib}_{kb}")
                            nc.tensor.transpose(tp[:], nat[:, 128 * k:128 * (k + 1)], eyer[:])
                            nc.vector.tensor_copy(seqT[kb][:, 128 * ib:128 * (ib + 1)], tp[:])

            if stage == "tpose":
                dummy_out(seqT[0][:, 0:1].bitcast(F32))

            # ---- phase 1: per head-pair block ----
            if stage in ("eb", "full"):
                s_t = [pp.tile([128, H], F32, tag=f"s_t{ib}", name=f"s_t{ib}") for ib in range(IB)]

                for eb in range(EB):
                    # aa projection for this e-block
                    aa_ps = pproj.tile([128, J], F32, tag="proj", name=f"aaps{eb}")
                    waa = wp.tile([128, KA * 128], F32R, tag="waa", name=f"waa{eb}")
                    nc.gpsimd.dma_start(
                        out=waa[:],
                        in_=aaw_ext.ap().rearrange("(kb p) e -> p kb e", p=128)[:, :, 128 * eb:128 * (eb + 1)])
                    for kb in range(KA):
                        nc.tensor.matmul(aa_ps[:], waa[:, 128 * kb:128 * (kb + 1)], aaT[kb][:],
                                         start=(kb == 0), stop=(kb == KA - 1))
                    aa_raw = wp.tile([128, J], F32, tag="araw", name=f"araw{eb}")
                    nc.vector.tensor_scalar_add(aa_raw[:], aa_ps[:], aab2[:, eb:eb + 1])
                    aa_sq = wp.tile([128, N], F32R, tag="seqsq", name=f"aasq{eb}")
                    nc.vector.tensor_mul(aa_sq[:, :J], aa_raw[:], aa_raw[:])
                    nsqa_ps = pb512.tile([2, J], F32, tag="b512", name=f"nsqa{eb}")
                    nc.tensor.matmul(nsqa_ps[:], pairmask[:], aa_sq[:, :J], start=True, stop=True)
                    nsqa_sb = wp.tile([2, J], F32, tag="nsq", bufs=3, name=f"nsqa_sb{eb}")
                    nc.vector.tensor_copy(nsqa_sb[:], nsqa_ps[:])
                    ibat_a = psm2.tile([128, 2 * JB], F32, tag="sm2", name=f"ibata{eb}")
                    for c in range(JB):
                        nc.tensor.transpose(ibat_a[:, 2 * c:2 * c + 2], nsqa_sb[:, 128 * c:128 * (c + 1)], eye2[:])
                    ya = rsqrt_newton(f"a{eb}", ibat_a[:, :2 * JB], 2 * JB, 1.0)
                    nc.vector.tensor_mul(ya[:, :2 * JB], ya[:, :2 * JB], maskT[:])
                    rowa_ps = pb512.tile([2, J], F32, tag="b512", name=f"rowa{eb}")
                    for c in range(JB):
                        nc.tensor.transpose(rowa_ps[:, 128 * c:128 * (c + 1)], ya[:, 2 * c:2 * c + 2], eye[:])
                    inva = wp.tile([2, J], F32R, tag="inva", name=f"inva{eb}")
                    nc.vector.tensor_copy(inva[:], rowa_ps[:])
                    bc_ps = pb512.tile([128, J], F32, tag="b512", name=f"bc{eb}")
                    nc.tensor.matmul(bc_ps[:], sel2[:], inva[:], start=True, stop=True)
                    aa_nrm = wp.tile([128, J], F32R, tag="aanrm", bufs=3, name=f"aanrm{eb}")
                    nc.vector.tensor_mul(aa_nrm[:], aa_raw[:], bc_ps[:])

                    # seq projection for this e-block
                    sq_ps = [pproj.tile([128, NH], F32, tag="proj", name=f"sqps{eb}_{c}") for c in range(2)]
                    for hf in range(2):
                        ws = wp.tile([128, (KS // 2) * 128], F32R, tag="wseq", name=f"ws{eb}_{hf}")
                        nc.gpsimd.dma_start(
                            out=ws[:],
                            in_=seqw_ext.ap().rearrange("(kb p) e -> p kb e", p=128)[:, (KS // 2) * hf:(KS // 2) * (hf + 1), 128 * eb:128 * (eb + 1)])
                        for k in range(KS // 2):
                            kb = hf * (KS // 2) + k
                            for c in range(2):
                                nc.tensor.matmul(sq_ps[c][:], ws[:, 128 * k:128 * (k + 1)], seqT[kb][:, NH * c:NH * (c + 1)],
                                                 start=(kb == 0), stop=(kb == KS - 1))
                    seq_sb = wp.tile([128, N], F32R, tag="seq", bufs=3, name=f"seqsb{eb}")
                    for c in range(2):
                        nc.vector.tensor_scalar_add(seq_sb[:, NH * c:NH * (c + 1)], sq_ps[c][:], seqb2[:, eb:eb + 1])
                    seq_sq = wp.tile([128, N], F32R, tag="seqsq", name=f"seqsq{eb}")
                    nc.vector.tensor_mul(seq_sq[:], seq_sb[:].bitcast(F32), seq_sb[:].bitcast(F32))
                    nsq_sb = wp.tile([2, N], F32, tag="nsq", bufs=3, name=f"nsq{eb}")
                    for c in range(2):
                        nsqs_ps = psm2.tile([2, NH], F32, tag="sm2", name=f"nsqs{eb}_{c}")
                        nc.tensor.matmul(nsqs_ps[:], pairmask[:], seq_sq[:, NH * c:NH * (c + 1)], start=True, stop=True)
                        nc.vector.tensor_copy(nsq_sb[:, NH * c:NH * (c + 1)], nsqs_ps[:])
                    ibat_s = psm2.tile([128, 2 * IB], F32, tag="sm2", name=f"ibats{eb}")
                    for ib in range(IB):
                        nc.tensor.transpose(ibat_s[:, 2 * ib:2 * ib + 2], nsq_sb[:, 128 * ib:128 * (ib + 1)], eye2[:])
                    invsT = rsqrt_newton(f"s{eb}", ibat_s[:, :2 * IB], 2 * IB, 1e-4)

                    # interactions + exp-accumulate for heads 2eb, 2eb+1
                    # hh inner: alternating head halves use disjoint PE row groups,
                    # letting the next LDWEIGHTS overlap the running matmul
                    for ib in range(IB):
                        for hh in range(2):
                            h = 2 * eb + hh
                            int_ps = pips.tile([128, J], F32, tag="ips", name=f"int{h}_{ib}")
                            nc.tensor.matmul(int_ps[:], seq_sb[64 * hh:64 * (hh + 1), 128 * ib:128 * (ib + 1)],
                                             aa_nrm[64 * hh:64 * (hh + 1), :], start=True, stop=True)
                            nc.scalar.activation(int_ps[:], int_ps[:], AF.Exp,
                                                 bias=cb[:, 0:1], scale=invsT[:, 2 * ib + hh:2 * ib + hh + 1],
                                                 accum_out=s_t[ib][:, h:h + 1])

            if stage == "eb":
                gating_tail()
                for ib in range(IB):
                    nc.sync.dma_start(out=out_ext[128 * ib:128 * (ib + 1)], in_=s_t[ib][:, 0:1])

            # ---- phase 2: finalize (batched per ACT function) ----
            if stage == "full":
                gating_tail()
                r1s = [wp.tile([128, H], F32, tag="r1", bufs=IB, name=f"r1_{ib}") for ib in range(IB)]
                pps = [wp.tile([128, 1], F32, tag="pp_t", bufs=IB, name=f"pp{ib}") for ib in range(IB)]
                for ib in range(IB):
                    nc.scalar.activation(r1s[ib][:], s_t[ib][:], AF.Ln, bias=cb[:, 1:2], scale=1.0)
                junk = wp.tile([128, H], F32, tag="junk", bufs=1, name="junk")
                for ib in range(IB):
                    nc.vector.tensor_scalar(r1s[ib][:], r1s[ib][:], 0.01, cvec[:, 0:1],
                                            op0=AL.mult, op1=AL.add)
                    nc.vector.tensor_mul(junk[:], r1s[ib][:], WV[:])
                    nc.vector.reduce_sum(pps[ib][:], junk[:], axis=mybir.AxisListType.X)
                    nc.vector.tensor_scalar_min(pps[ib][:], pps[ib][:], 80.0)
                for ib in range(IB):
                    nc.scalar.activation(pps[ib][:], pps[ib][:], AF.Exp, bias=cb[:, 2:3], scale=1.0)
                for ib in range(IB):
                    nc.scalar.activation(pps[ib][:], pps[ib][:], AF.Ln, bias=1.0, scale=1.0)
                    nc.sync.dma_start(out=out_ext[128 * ib:128 * (ib + 1)], in_=pps[ib][:])

    nc.compile()
    _GRAPH_CACHE[key] = nc
    return nc


def _prep_in_maps(inputs):
    seq_embed = np.ascontiguousarray(inputs["seq_embed"], dtype=np.float32)
    aa_embed = np.ascontiguousarray(inputs["aa_embed"], dtype=np.float32)
    ctx = np.ascontiguousarray(inputs["contextual_embed"], dtype=np.float32)
    aa_mask = np.asarray(inputs["aa_mask"])
    seq_w = np.ascontiguousarray(inputs["seq_w"], dtype=np.float32)
    seq_b = np.asarray(inputs["seq_b"], dtype=np.float32)
    aa_w = np.ascontiguousarray(inputs["aa_w"], dtype=np.float32)
    aa_b = np.asarray(inputs["aa_b"], dtype=np.float32)
    tlw = np.asarray(inputs["to_logits_w"], dtype=np.float32)
    ctx_w = np.asarray(inputs["ctx_w"], dtype=np.float32)
    ctx_b = np.asarray(inputs["ctx_b"], dtype=np.float32)
    pred_w = np.ascontiguousarray(inputs["pred_w"], dtype=np.float32)

    # permute gating space from h-major (h*32+e) to e-major (e*32+h)
    perm = (np.arange(H * H).reshape(H, H).T).reshape(-1)  # new[e*32+h] = old[h*32+e]
    ctx_wp = np.ascontiguousarray(ctx_w[:, perm])
    ctx_bp = np.ascontiguousarray(ctx_b[perm])[None, :]
    tlwT = np.ascontiguousarray(tlw.T.reshape(1, H * H))   # [1,(e h)]

    seq_b2 = np.ascontiguousarray(seq_b.reshape(EB, 128).T)
    aa_b2 = np.ascontiguousarray(aa_b.reshape(EB, 128).T)
    eye128 = np.eye(128, dtype=np.float32)
    eye2 = np.eye(2, dtype=np.float32)
    pairmask = np.zeros((128, 2), dtype=np.float32)
    pairmask[:64, 0] = 1.0
    pairmask[64:, 1] = 1.0
    sel2 = np.zeros((2, 128), dtype=np.float32)
    sel2[0, :64] = 1.0
    sel2[1, 64:] = 1.0
    ones1 = np.ones((1, 128), dtype=np.float32)

    in_maps = []
    for b in range(B):
        m = aa_mask[b].astype(np.float32)
        n_b = max(float(m.sum()), 1.0)
        cval = 0.01 * (40.0 - 2.0 * math.log(n_b))  # reference's logavgexp subtracts ln n twice
        mT = np.zeros((128, 2 * JB), dtype=np.float32)
        for c in range(JB):
            mT[:, 2 * c] = m[128 * c:128 * (c + 1)]
            mT[:, 2 * c + 1] = m[128 * c:128 * (c + 1)]
        in_maps.append({
            "seq": seq_embed[b],
            "aa": aa_embed[b],
            "seq_w": seq_w,
            "aa_w": aa_w,
            "seq_b2": seq_b2,
            "aa_b2": aa_b2,
            "ctxT": np.ascontiguousarray(ctx[b].reshape(CTX_D // 128, 128).T),
            "ctx_wp": ctx_wp,
            "ctx_bp": ctx_bp,
            "tlwT": tlwT,
            "pred_w": pred_w,
            "maskT": mT,
            "cvec": np.full((128, 1), cval, dtype=np.float32),
            "eye128": eye128,
            "eye2": eye2,
            "pairmask": pairmask,
            "sel2": sel2,
            "ones1": ones1,
        })
    return in_maps


def _run(inputs, trace=False, stage="full", n_cores=B):
    from concourse.bass_utils import run_bass_kernel_spmd
    pred_b_val = float(np.asarray(inputs["pred_b"]).reshape(-1)[0])
    nc = _build(pred_b_val, stage=stage)
    in_maps = _prep_in_maps(inputs)
    res = run_bass_kernel_spmd(nc, in_maps[:n_cores], core_ids=list(range(n_cores)), trace=trace)
    out = np.stack([res.results[c]["out"] for c in range(n_cores)], axis=0)
    return out, res


def kernel(**inputs) -> np.ndarray:
    out, _ = _run(inputs, trace=False)
    return out
